# revision 4
# baseline (speedup 1.0000x reference)
"""AdaptiveGCN Trainium2 kernel — 8-core data-parallel over the graph/batch dim.

Layout strategy (per core, channel-major activations):
  - activations live as [C, rows] (channel on partitions), weights as [cin, cout].
  - channel matmul uses the row-major-out form: out[rows, cout] = lhsT(act).T @ rhs(W)
  - message passing uses the channel-major-out form:
      agg[c, i] = lhsT(hm[rows_j, c]).T @ rhs(A_blockdiag[rows_j, rows_i])
    so the layer output lands back in channel-major with zero transposes.
  - channel matmuls run as float32r (full PE rate at free-dim >= 256),
    message-passing matmuls run bf16 (tiny free dim).
  - BatchNorm is training-mode over ALL rows: local Welford stats via
    bn_stats/bn_aggr, merged across the 8 cores with one small AllReduce
    per layer. Activations round-trip HBM (f32) across the BN barriers.
"""

import numpy as np
import ml_dtypes

# problem dims (hardcoded per task spec)
B = 4096
N = 21
CIN = 512
H1 = 1024
H2 = 1024
COUT = 512
FCO = 1024
SEH = 128
RANK = 10
EPS = 1e-5
THRESH = 0.1
NCORES = 8
GPC = B // NCORES            # graphs per core
ST_G = 18                    # graphs per supertile (378 rows)
SUB_G = 6                    # graphs per matmul row-tile (126 rows)


def _host_adjacency(nodevec1, nodevec2):
    """Replicate the reference adjacency computation in f32 numpy."""
    nv1 = np.asarray(nodevec1, np.float32)
    nv2 = np.asarray(nodevec2, np.float32)
    logits = (nv1 @ nv2).astype(np.float32)
    adp = (1.0 / (1.0 + np.exp(-logits, dtype=np.float32))).astype(np.float32)
    at = (adp + np.eye(N, dtype=np.float32)).astype(np.float32)
    deg = at.sum(1, dtype=np.float32)
    dinv = np.where(deg > 0, deg.astype(np.float32) ** -0.5, 0.0).astype(np.float32)
    A = (dinv[:, None] * at * dinv[None, :]).astype(np.float32)
    return np.where(A > THRESH, A, 0.0).astype(np.float32)


def _pack_ch(vec, nch):
    """[nch*128] channel vector -> [128, nch] (partition = channel % 128)."""
    v = np.asarray(vec, np.float32).reshape(nch, 128)
    return np.ascontiguousarray(v.T)


def _supertiles(gpc):
    out, g = [], 0
    while g < gpc:
        gc = min(ST_G, gpc - g)
        out.append((g, gc))
        g += gc
    return out


def _subtiles(gc):
    out, r, rows = [], 0, gc * N
    while r < rows:
        rr = min(SUB_G * N, rows - r)
        out.append((r, rr))
        r += rr
    return out


def build_nc(gpc=GPC, n_cores=NCORES):
    import concourse.bass as bass
    import concourse.bacc as bacc
    import concourse.mybir as mybir
    import concourse.tile as tile

    f32 = mybir.dt.float32
    f32r = mybir.dt.float32r
    bf16 = mybir.dt.bfloat16
    AF = mybir.ActivationFunctionType
    OP = mybir.AluOpType
    AX = mybir.AxisListType.X

    ROWS = gpc * N
    STR = ST_G * N               # supertile rows (378)
    sts = _supertiles(gpc)
    n_st = len(sts)
    rg = [list(range(n_cores))]
    
    nc = bacc.Bacc("TRN2", target_bir_lowering=False, debug=False,
                   num_devices=n_cores)

    # ---- external I/O -------------------------------------------------
    xT = nc.dram_tensor("xT", [CIN, ROWS], f32r, kind="ExternalInput")
    w1_d = nc.dram_tensor("w1", [CIN, H1], f32r, kind="ExternalInput")
    rs1_d = nc.dram_tensor("rs1w", [CIN, H1], f32r, kind="ExternalInput")
    w2_d = nc.dram_tensor("w2", [H1, H2], f32r, kind="ExternalInput")
    w3_d = nc.dram_tensor("w3", [H2, COUT], f32r, kind="ExternalInput")
    fcw_d = nc.dram_tensor("fcw", [COUT, FCO], f32r, kind="ExternalInput")
    sew1_d = nc.dram_tensor("sew1", [CIN, SEH], f32r, kind="ExternalInput")  # pre-scaled 1/21
    sew2_d = nc.dram_tensor("sew2", [SEH, CIN], f32r, kind="ExternalInput")
    abd_d = nc.dram_tensor("abd", [SUB_G * N, SUB_G * N], bf16, kind="ExternalInput")
    pk_d = {}
    for nm, nch in [("b1p", 8), ("bn1gp", 8), ("bn1bp", 8), ("rs1bp", 8),
                    ("b2p", 8), ("bn2gp", 8), ("bn2bp", 8),
                    ("b3p", 4), ("bn3gp", 4), ("bn3bp", 4), ("fcbp", 8)]:
        pk_d[nm] = nc.dram_tensor(nm, [128, nch], f32, kind="ExternalInput")
    out_d = nc.dram_tensor("out", [FCO, gpc], f32, kind="ExternalOutput")

    with tile.TileContext(nc) as tc:
        with tc.tile_pool(name="persist", bufs=1) as pp, \
             tc.tile_pool(name="scratch", bufs=2) as scp, \
             tc.tile_pool(name="dram", bufs=1, space="DRAM") as dmp, \
             tc.tile_pool(name="ps_hm", bufs=2, space="PSUM") as ps_hm, \
             tc.tile_pool(name="ps_small", bufs=3, space="PSUM") as ps_s:

            # DRAM spill buffers (pool tiles so Tile tracks deps), all f32
            z1_t = dmp.tile([H1, ROWS], f32, tag="z1")
            r1_t = dmp.tile([H1, ROWS], f32, tag="r1")
            h1_t = dmp.tile([H1, ROWS], f32r, tag="h1sp")
            z2_t = dmp.tile([H2, ROWS], f32, tag="z2")
            stb_in = {l: dmp.tile([128, 2 * nch], f32, tag=f"stbi{l}", name=f"stbi{l}")
                      for l, nch in [(1, 8), (2, 8), (3, 4)]}
            stb_out = {l: dmp.tile([128, 2 * nch], f32, tag=f"stbo{l}", name=f"stbo{l}")
                       for l, nch in [(1, 8), (2, 8), (3, 4)]}

            # packed per-channel params
            pk = {}
            for nm, d in pk_d.items():
                t = pp.tile(list(d.shape), f32, tag=nm, name=nm)
                nc.sync.dma_start(t[:], d.ap()[:])
                pk[nm] = t
            abd_sb = pp.tile([SUB_G * N, SUB_G * N], bf16, tag="abd")
            nc.sync.dma_start(abd_sb[:], abd_d.ap()[:])

            # bn stat slots + pooled accumulators + affine params
            bnsl = {1: [pp.tile([128, n_st * 6], f32, tag=f"bnsl1_{c}", name=f"bnsl1_{c}") for c in range(8)],
                    2: [pp.tile([128, n_st * 6], f32, tag=f"bnsl2_{c}", name=f"bnsl2_{c}") for c in range(8)],
                    3: [pp.tile([128, n_st * 6], f32, tag=f"bnsl3_{c}", name=f"bnsl3_{c}") for c in range(4)]}
            pool_sb = [pp.tile([128, gpc], f32, tag=f"pool_{c}", name=f"pool_{c}") for c in range(4)]
            sv = {1: pp.tile([128, 8], f32, tag="s1v", name="s1v"),
                  2: pp.tile([128, 8], f32, tag="s2v", name="s2v"),
                  3: pp.tile([128, 4], f32, tag="s3v", name="s3v")}
            tv = {1: pp.tile([128, 8], f32, tag="t1v", name="t1v"),
                  2: pp.tile([128, 8], f32, tag="t2v", name="t2v"),
                  3: pp.tile([128, 4], f32, tag="t3v", name="t3v")}
            s3p = pp.tile([128, 4], f32, tag="s3p")   # s3 / 21 for pooled affine

            def finalize_stats(l, nch, gp, bp, extra_bias=None):
                """bn slots -> AllReduce -> affine params sv[l], tv[l]."""
                loc = scp.tile([128, 2 * nch], f32, tag=f"loc{l}", name=f"loc{l}")
                for c in range(nch):
                    ag = scp.tile([128, 2], f32, tag="bnag", name="bnag")
                    nc.vector.bn_aggr(ag[:], bnsl[l][c][:])
                    nc.vector.tensor_copy(loc[:, c:c + 1], ag[:, 0:1])
                    # Ez2 = mean*mean + var
                    nc.vector.scalar_tensor_tensor(
                        loc[:, nch + c:nch + c + 1], ag[:, 0:1], ag[:, 0:1],
                        ag[:, 1:2], op0=OP.mult, op1=OP.add)
                nc.sync.dma_start(stb_in[l][:], loc[:])
                nc.gpsimd.collective_compute(
                    "AllReduce", OP.add, replica_groups=rg,
                    ins=[stb_in[l][:].opt()], outs=[stb_out[l][:].opt()])
                red = scp.tile([128, 2 * nch], f32, tag=f"red{l}", name=f"red{l}")
                nc.sync.dma_start(red[:], stb_out[l][:])
                mg = scp.tile([128, nch], f32, tag=f"mg{l}", name=f"mg{l}")
                e2 = scp.tile([128, nch], f32, tag=f"e2{l}", name=f"e2{l}")
                nc.vector.tensor_scalar_mul(mg[:], red[:, :nch], 1.0 / n_cores)
                nc.vector.tensor_scalar_mul(e2[:], red[:, nch:], 1.0 / n_cores)
                var = scp.tile([128, nch], f32, tag=f"var{l}", name=f"var{l}")
                nc.vector.tensor_tensor(var[:], mg[:], mg[:], op=OP.mult)
                nc.vector.tensor_tensor(var[:], e2[:], var[:], op=OP.subtract)
                nc.vector.tensor_scalar_add(var[:], var[:], EPS)
                sq = scp.tile([128, nch], f32, tag=f"sq{l}", name=f"sq{l}")
                nc.scalar.activation(sq[:], var[:], AF.Sqrt)
                y0 = scp.tile([128, nch], f32, tag=f"y0{l}", name=f"y0{l}")
                nc.vector.reciprocal(y0[:], sq[:])
                # one Newton step: y1 = y0 * (1.5 - 0.5 * var * y0^2)
                yy = scp.tile([128, nch], f32, tag=f"yy{l}", name=f"yy{l}")
                nc.vector.tensor_tensor(yy[:], y0[:], y0[:], op=OP.mult)
                nc.vector.tensor_tensor(yy[:], var[:], yy[:], op=OP.mult)
                nc.vector.tensor_scalar(yy[:], yy[:], -0.5, 1.5,
                                        op0=OP.mult, op1=OP.add)
                nc.vector.tensor_tensor(y0[:], y0[:], yy[:], op=OP.mult)
                nc.vector.tensor_tensor(sv[l][:, :nch], gp[:], y0[:], op=OP.mult)
                ms = scp.tile([128, nch], f32, tag=f"ms{l}", name=f"ms{l}")
                nc.vector.tensor_tensor(ms[:], mg[:], sv[l][:, :nch], op=OP.mult)
                nc.vector.tensor_tensor(tv[l][:, :nch], bp[:], ms[:], op=OP.subtract)
                if extra_bias is not None:
                    nc.vector.tensor_tensor(tv[l][:, :nch], tv[l][:, :nch],
                                            extra_bias[:], op=OP.add)

            # ================= Phase A: SE gate + layer 1 + residual ====
            with tc.tile_pool(name="wA", bufs=1) as wp, \
                 tc.tile_pool(name="actA", bufs=2) as ap_:
                w1_sb = []
                rs1_sb = []
                sew1_sb = []
                for k in range(4):
                    t = wp.tile([128, H1], f32r, tag=f"w1_{k}", name=f"w1sb_{k}")
                    nc.sync.dma_start(t[:], w1_d.ap()[k * 128:(k + 1) * 128, :])
                    w1_sb.append(t)
                    t = wp.tile([128, H1], f32r, tag=f"rs1_{k}", name=f"rs1sb_{k}")
                    nc.sync.dma_start(t[:], rs1_d.ap()[k * 128:(k + 1) * 128, :])
                    rs1_sb.append(t)
                    t = wp.tile([128, SEH], f32r, tag=f"sew1_{k}", name=f"sew1sb_{k}")
                    nc.sync.dma_start(t[:], sew1_d.ap()[k * 128:(k + 1) * 128, :])
                    sew1_sb.append(t)
                sew2_sb = wp.tile([128, CIN], f32r, tag="sew2")
                nc.sync.dma_start(sew2_sb[:], sew2_d.ap()[:])

                for sti, (g0, gc) in enumerate(sts):
                    rows = gc * N
                    c0 = g0 * N
                    x_sb = []
                    for k in range(4):
                        t = ap_.tile([128, STR], f32r, tag=f"x_{k}", name=f"x_{k}")
                        nc.sync.dma_start(t[:, :rows],
                                          xT.ap()[k * 128:(k + 1) * 128, c0:c0 + rows])
                        x_sb.append(t)
                    # SE: node-sum (1/21 folded into sew1) -> 2 tiny mms
                    xm = []
                    for k in range(4):
                        t = ap_.tile([128, ST_G], f32r, tag=f"xm_{k}", name=f"xm_{k}")
                        with nc.allow_low_precision(reason="fp32r rounding for SE matmul"):
                            nc.vector.tensor_reduce(
                                t[:, :gc],
                                x_sb[k][:, :rows].rearrange("p (g n) -> p g n", n=N),
                                axis=AX, op=OP.add)
                        xm.append(t)
                    y1ps = ps_s.tile([128, STR], f32, tag="small", name="y1ps")
                    for k in range(4):
                        nc.tensor.matmul(y1ps[:, :gc], lhsT=sew1_sb[k][:],
                                         rhs=xm[k][:, :gc],
                                         start=(k == 0), stop=(k == 3))
                    y1_sb = ap_.tile([128, ST_G], f32r, tag="y1", name="y1")
                    nc.scalar.activation(y1_sb[:, :gc], y1ps[:, :gc], AF.Relu)
                    y_sb = []
                    for m in range(4):
                        y2ps = ps_s.tile([128, STR], f32, tag="small", name="y2ps")
                        nc.tensor.matmul(y2ps[:, :gc],
                                         lhsT=sew2_sb[:, m * 128:(m + 1) * 128],
                                         rhs=y1_sb[:, :gc], start=True, stop=True)
                        t = ap_.tile([128, ST_G], f32, tag=f"y_{m}", name=f"y_{m}")
                        nc.scalar.activation(t[:, :gc], y2ps[:, :gc], AF.Sigmoid)
                        y_sb.append(t)
                    # gate
                    xg = []
                    for k in range(4):
                        t = ap_.tile([128, STR], f32r, tag=f"xg_{k}", name=f"xg_{k}")
                        nc.vector.tensor_tensor(
                            t[:, :rows].rearrange("p (g n) -> p g n", n=N),
                            x_sb[k][:, :rows].rearrange("p (g n) -> p g n", n=N),
                            y_sb[k][:, :gc].broadcast_to([128, gc, N]),
                            op=OP.mult)
                        xg.append(t)
                    # layer-1 matmul + message passing
                    z1sb = [ap_.tile([128, STR], f32, tag=f"z1sb_{c}", name=f"z1sb_{c}")
                            for c in range(8)]
                    for (r0, rr) in _subtiles(gc):
                        hmps = ps_hm.tile([128, H1], f32, tag="hm", name="hmps")
                        for k in range(4):
                            for n2 in range(2):
                                nc.tensor.matmul(
                                    hmps[:rr, n2 * 512:(n2 + 1) * 512],
                                    lhsT=xg[k][:, r0:r0 + rr],
                                    rhs=w1_sb[k][:, n2 * 512:(n2 + 1) * 512],
                                    start=(k == 0), stop=(k == 3))
                        hm_sb = ap_.tile([128, H1], bf16, tag="hm_sb", name="hm_sb")
                        nc.vector.tensor_copy(hm_sb[:rr, :], hmps[:rr, :])
                        for c in range(8):
                            aggps = ps_s.tile([128, STR], f32, tag="small", name="aggps")
                            nc.tensor.matmul(aggps[:, :rr],
                                             lhsT=hm_sb[:rr, c * 128:(c + 1) * 128],
                                             rhs=abd_sb[:rr, :rr],
                                             start=True, stop=True)
                            nc.scalar.activation(z1sb[c][:, r0:r0 + rr],
                                                 aggps[:, :rr], AF.Identity,
                                                 bias=pk["b1p"][:, c:c + 1])
                    for c in range(8):
                        nc.sync.dma_start(z1_t[c * 128:(c + 1) * 128, c0:c0 + rows],
                                          z1sb[c][:, :rows])
                        nc.vector.bn_stats(bnsl[1][c][:, sti * 6:(sti + 1) * 6],
                                           z1sb[c][:, :rows])
                    # residual r1 = xg @ rs1_w (channel-major-out)
                    for c in range(8):
                        r1ps = ps_s.tile([128, STR], f32, tag="small", name="r1ps")
                        for k in range(4):
                            nc.tensor.matmul(r1ps[:, :rows],
                                             lhsT=rs1_sb[k][:, c * 128:(c + 1) * 128],
                                             rhs=xg[k][:, :rows],
                                             start=(k == 0), stop=(k == 3))
                        r1sb = ap_.tile([128, STR], f32, tag="r1sb", name="r1sb")
                        nc.vector.tensor_copy(r1sb[:, :rows], r1ps[:, :rows])
                        nc.sync.dma_start(r1_t[c * 128:(c + 1) * 128, c0:c0 + rows],
                                          r1sb[:, :rows])

                finalize_stats(1, 8, pk["bn1gp"], pk["bn1bp"], pk["rs1bp"])

            # ================= Phase B: junction 1 + layer 2 ============
            with tc.tile_pool(name="wB", bufs=1) as wp, \
                 tc.tile_pool(name="actB", bufs=2) as ap_:
                w2_sb = []
                for k in range(8):
                    t = wp.tile([128, H2], f32r, tag=f"w2_{k}", name=f"w2sb_{k}")
                    nc.sync.dma_start(t[:], w2_d.ap()[k * 128:(k + 1) * 128, :])
                    w2_sb.append(t)
                for sti, (g0, gc) in enumerate(sts):
                    rows = gc * N
                    c0 = g0 * N
                    h1 = []
                    for c in range(8):
                        z1r = ap_.tile([128, STR], f32, tag=f"z1r_{c}", name=f"z1r_{c}")
                        nc.sync.dma_start(z1r[:, :rows],
                                          z1_t[c * 128:(c + 1) * 128, c0:c0 + rows])
                        r1r = ap_.tile([128, STR], f32, tag=f"r1r_{c}", name=f"r1r_{c}")
                        nc.sync.dma_start(r1r[:, :rows],
                                          r1_t[c * 128:(c + 1) * 128, c0:c0 + rows])
                        ut = ap_.tile([128, STR], f32, tag=f"ut_{c}", name=f"ut_{c}")
                        nc.vector.scalar_tensor_tensor(
                            ut[:, :rows], z1r[:, :rows], sv[1][:, c:c + 1],
                            r1r[:, :rows], op0=OP.mult, op1=OP.add)
                        t = ap_.tile([128, STR], f32r, tag=f"h1_{c}", name=f"h1_{c}")
                        nc.scalar.activation(t[:, :rows], ut[:, :rows], AF.Relu,
                                             bias=tv[1][:, c:c + 1])
                        nc.sync.dma_start(h1_t[c * 128:(c + 1) * 128, c0:c0 + rows],
                                          t[:, :rows])
                        h1.append(t)
                    z2sb = [ap_.tile([128, STR], f32, tag=f"z2sb_{c}", name=f"z2sb_{c}")
                            for c in range(8)]
                    for (r0, rr) in _subtiles(gc):
                        hmps = ps_hm.tile([128, H2], f32, tag="hm", name="hmps2")
                        for k in range(8):
                            for n2 in range(2):
                                nc.tensor.matmul(
                                    hmps[:rr, n2 * 512:(n2 + 1) * 512],
                                    lhsT=h1[k][:, r0:r0 + rr],
                                    rhs=w2_sb[k][:, n2 * 512:(n2 + 1) * 512],
                                    start=(k == 0), stop=(k == 7))
                        hm_sb = ap_.tile([128, H2], bf16, tag="hm_sb", name="hm_sb2")
                        nc.vector.tensor_copy(hm_sb[:rr, :], hmps[:rr, :])
                        for c in range(8):
                            aggps = ps_s.tile([128, STR], f32, tag="small", name="aggps2")
                            nc.tensor.matmul(aggps[:, :rr],
                                             lhsT=hm_sb[:rr, c * 128:(c + 1) * 128],
                                             rhs=abd_sb[:rr, :rr],
                                             start=True, stop=True)
                            nc.scalar.activation(z2sb[c][:, r0:r0 + rr],
                                                 aggps[:, :rr], AF.Identity,
                                                 bias=pk["b2p"][:, c:c + 1])
                    for c in range(8):
                        nc.sync.dma_start(z2_t[c * 128:(c + 1) * 128, c0:c0 + rows],
                                          z2sb[c][:, :rows])
                        nc.vector.bn_stats(bnsl[2][c][:, sti * 6:(sti + 1) * 6],
                                           z2sb[c][:, :rows])
                finalize_stats(2, 8, pk["bn2gp"], pk["bn2bp"])

            # ================= Phase C: junction 2 + layer 3 + pool =====
            with tc.tile_pool(name="wC", bufs=1) as wp, \
                 tc.tile_pool(name="actC", bufs=2) as ap_:
                w3_sb = []
                for k in range(8):
                    t = wp.tile([128, COUT], f32r, tag=f"w3_{k}", name=f"w3sb_{k}")
                    nc.sync.dma_start(t[:], w3_d.ap()[k * 128:(k + 1) * 128, :])
                    w3_sb.append(t)
                for sti, (g0, gc) in enumerate(sts):
                    rows = gc * N
                    c0 = g0 * N
                    h2 = []
                    for c in range(8):
                        z2r = ap_.tile([128, STR], f32, tag=f"z2r_{c}", name=f"z2r_{c}")
                        nc.sync.dma_start(z2r[:, :rows],
                                          z2_t[c * 128:(c + 1) * 128, c0:c0 + rows])
                        h1r = ap_.tile([128, STR], f32r, tag=f"h1r_{c}", name=f"h1r_{c}")
                        nc.sync.dma_start(h1r[:, :rows],
                                          h1_t[c * 128:(c + 1) * 128, c0:c0 + rows])
                        bt = ap_.tile([128, STR], f32, tag=f"bt_{c}", name=f"bt_{c}")
                        nc.vector.scalar_tensor_tensor(
                            bt[:, :rows], z2r[:, :rows], sv[2][:, c:c + 1],
                            h1r[:, :rows], op0=OP.mult, op1=OP.add)
                        t = ap_.tile([128, STR], f32r, tag=f"h2_{c}", name=f"h2_{c}")
                        nc.scalar.activation(t[:, :rows], bt[:, :rows], AF.Relu,
                                             bias=tv[2][:, c:c + 1])
                        h2.append(t)
                    z3sb = [ap_.tile([128, STR], f32, tag=f"z3sb_{c}", name=f"z3sb_{c}")
                            for c in range(4)]
                    for (r0, rr) in _subtiles(gc):
                        hmps = ps_hm.tile([128, H2], f32, tag="hm", name="hmps3")
                        for k in range(8):
                            nc.tensor.matmul(hmps[:rr, :COUT],
                                             lhsT=h2[k][:, r0:r0 + rr],
                                             rhs=w3_sb[k][:],
                                             start=(k == 0), stop=(k == 7))
                        hm_sb = ap_.tile([128, H2], bf16, tag="hm_sb", name="hm_sb3")
                        nc.vector.tensor_copy(hm_sb[:rr, :COUT], hmps[:rr, :COUT])
                        for c in range(4):
                            aggps = ps_s.tile([128, STR], f32, tag="small", name="aggps3")
                            nc.tensor.matmul(aggps[:, :rr],
                                             lhsT=hm_sb[:rr, c * 128:(c + 1) * 128],
                                             rhs=abd_sb[:rr, :rr],
                                             start=True, stop=True)
                            nc.scalar.activation(z3sb[c][:, r0:r0 + rr],
                                                 aggps[:, :rr], AF.Identity,
                                                 bias=pk["b3p"][:, c:c + 1])
                    for c in range(4):
                        nc.vector.bn_stats(bnsl[3][c][:, sti * 6:(sti + 1) * 6],
                                           z3sb[c][:, :rows])
                        nc.vector.tensor_reduce(
                            pool_sb[c][:, g0:g0 + gc],
                            z3sb[c][:, :rows].rearrange("p (g n) -> p g n", n=N),
                            axis=AX, op=OP.add)
                finalize_stats(3, 4, pk["bn3gp"], pk["bn3bp"])
                nc.vector.tensor_scalar_mul(s3p[:], sv[3][:], 1.0 / N)

            # ================= Phase D: pooled affine + FC ==============
            with tc.tile_pool(name="wD", bufs=1) as wp, \
                 tc.tile_pool(name="actD", bufs=2) as ap_:
                fcw_sb = []
                for k in range(4):
                    t = wp.tile([128, FCO], f32r, tag=f"fcw_{k}", name=f"fcwsb_{k}")
                    nc.sync.dma_start(t[:], fcw_d.ap()[k * 128:(k + 1) * 128, :])
                    fcw_sb.append(t)
                pbn = []
                for c in range(4):
                    t = ap_.tile([128, gpc], f32r, tag=f"pbn_{c}", name=f"pbn_{c}")
                    nc.vector.tensor_scalar(t[:], pool_sb[c][:],
                                            s3p[:, c:c + 1], tv[3][:, c:c + 1],
                                            op0=OP.mult, op1=OP.add)
                    pbn.append(t)
                for m in range(8):
                    fcps = ps_s.tile([128, max(gpc, STR)], f32, tag="small", name="fcps")
                    for k in range(4):
                        nc.tensor.matmul(fcps[:, :gpc],
                                         lhsT=fcw_sb[k][:, m * 128:(m + 1) * 128],
                                         rhs=pbn[k][:],
                                         start=(k == 0), stop=(k == 3))
                    osb = ap_.tile([128, gpc], f32, tag="osb", name="osb")
                    nc.scalar.activation(osb[:], fcps[:, :gpc], AF.Identity,
                                         bias=pk["fcbp"][:, m:m + 1])
                    nc.sync.dma_start(out_d.ap()[m * 128:(m + 1) * 128, :], osb[:])

    nc.compile()
    return nc


def host_prep(inputs, gpc=GPC, n_cores=NCORES):
    """Build per-core in_maps from the full problem inputs."""
    g = lambda k: np.asarray(inputs[k], np.float32)
    A = _host_adjacency(inputs["nodevec1"], inputs["nodevec2"])
    R = SUB_G * N
    abd = np.zeros((R, R), np.float32)
    for b in range(SUB_G):
        abd[b * N:(b + 1) * N, b * N:(b + 1) * N] = A.T   # rhs[j, i] = A[i, j]
    abd = abd.astype(ml_dtypes.bfloat16)

    shared = {
        "w1": g("W1"), "rs1w": g("rs1_w"), "w2": g("W2"), "w3": g("W3"),
        "fcw": g("fc_w"),
        "sew1": (g("se_w1") / np.float32(N)).astype(np.float32),
        "sew2": g("se_w2"), "abd": abd,
        "b1p": _pack_ch(g("b1"), 8), "bn1gp": _pack_ch(g("bn1_g"), 8),
        "bn1bp": _pack_ch(g("bn1_b"), 8), "rs1bp": _pack_ch(g("rs1_b"), 8),
        "b2p": _pack_ch(g("b2"), 8), "bn2gp": _pack_ch(g("bn2_g"), 8),
        "bn2bp": _pack_ch(g("bn2_b"), 8),
        "b3p": _pack_ch(g("b3"), 4), "bn3gp": _pack_ch(g("bn3_g"), 4),
        "bn3bp": _pack_ch(g("bn3_b"), 4), "fcbp": _pack_ch(g("fc_b"), 8),
    }
    shared = {k: np.ascontiguousarray(v) for k, v in shared.items()}
    x = g("x")
    rows = gpc * N
    in_maps = []
    for i in range(n_cores):
        m = dict(shared)
        m["xT"] = np.ascontiguousarray(x[i * rows:(i + 1) * rows, :].T)
        in_maps.append(m)
    return in_maps


_cache = {}


def run(inputs, trace=False, trace_cores=None):
    from concourse.bass_utils import run_bass_kernel_spmd
    key = (GPC, NCORES)
    if key not in _cache:
        _cache[key] = build_nc(GPC, NCORES)
    nc = _cache[key]
    in_maps = host_prep(inputs, GPC, NCORES)
    res = run_bass_kernel_spmd(nc, in_maps, core_ids=list(range(NCORES)),
                               trace=trace, trace_cores=trace_cores)
    shards = [np.asarray(res.results[i]["out"]) for i in range(NCORES)]
    out = np.concatenate([s.T for s in shards], axis=0).astype(np.float32)
    return out, res


def kernel(**inputs) -> np.ndarray:
    out, _ = run(inputs, trace=False)
    return out


# revision 5
# speedup vs baseline: 1.0873x; 1.0873x over previous
"""AdaptiveGCN Trainium2 kernel — 8-core data-parallel over the graph/batch dim.

Layout strategy (per core, channel-major activations):
  - activations live as [C, rows] (channel on partitions), weights as [cin, cout].
  - channel matmul uses the row-major-out form: out[rows, cout] = lhsT(act).T @ rhs(W)
  - message passing uses the channel-major-out form:
      agg[c, i] = lhsT(hm[rows_j, c]).T @ rhs(A_blockdiag[rows_j, rows_i])
    so the layer output lands back in channel-major with zero transposes.
  - channel matmuls run as float32r (full PE rate at free-dim >= 256),
    message-passing matmuls run bf16 (tiny free dim).
  - BatchNorm is training-mode over ALL rows: local Welford stats via
    bn_stats/bn_aggr, merged across the 8 cores with one small AllReduce
    per layer. Activations round-trip HBM (f32) across the BN barriers.
"""

import numpy as np
import ml_dtypes

# problem dims (hardcoded per task spec)
B = 4096
N = 21
CIN = 512
H1 = 1024
H2 = 1024
COUT = 512
FCO = 1024
SEH = 128
RANK = 10
EPS = 1e-5
THRESH = 0.1
NCORES = 8
GPC = B // NCORES            # graphs per core
ST_G = 18                    # graphs per supertile (378 rows)
SUB_G = 6                    # graphs per matmul row-tile (126 rows)


def _host_adjacency(nodevec1, nodevec2):
    """Replicate the reference adjacency computation in f32 numpy."""
    nv1 = np.asarray(nodevec1, np.float32)
    nv2 = np.asarray(nodevec2, np.float32)
    logits = (nv1 @ nv2).astype(np.float32)
    adp = (1.0 / (1.0 + np.exp(-logits, dtype=np.float32))).astype(np.float32)
    at = (adp + np.eye(N, dtype=np.float32)).astype(np.float32)
    deg = at.sum(1, dtype=np.float32)
    dinv = np.where(deg > 0, deg.astype(np.float32) ** -0.5, 0.0).astype(np.float32)
    A = (dinv[:, None] * at * dinv[None, :]).astype(np.float32)
    return np.where(A > THRESH, A, 0.0).astype(np.float32)


def _pack_ch(vec, nch):
    """[nch*128] channel vector -> [128, nch] (partition = channel % 128)."""
    v = np.asarray(vec, np.float32).reshape(nch, 128)
    return np.ascontiguousarray(v.T)


def _supertiles(gpc):
    out, g = [], 0
    while g < gpc:
        gc = min(ST_G, gpc - g)
        out.append((g, gc))
        g += gc
    return out


def _subtiles(gc):
    out, r, rows = [], 0, gc * N
    while r < rows:
        rr = min(SUB_G * N, rows - r)
        out.append((r, rr))
        r += rr
    return out


def build_nc(gpc=GPC, n_cores=NCORES):
    import concourse.bass as bass
    import concourse.bacc as bacc
    import concourse.mybir as mybir
    import concourse.tile as tile

    f32 = mybir.dt.float32
    f32r = mybir.dt.float32r
    bf16 = mybir.dt.bfloat16
    f16 = mybir.dt.float16
    AF = mybir.ActivationFunctionType
    OP = mybir.AluOpType
    AX = mybir.AxisListType.X

    ROWS = gpc * N
    STR = ST_G * N               # supertile rows (378)
    sts = _supertiles(gpc)
    n_st = len(sts)
    rg = [list(range(n_cores))]
    
    nc = bacc.Bacc("TRN2", target_bir_lowering=False, debug=False,
                   num_devices=n_cores)

    # ---- external I/O -------------------------------------------------
    xT = nc.dram_tensor("xT", [CIN, ROWS], f32r, kind="ExternalInput")
    w1_d = nc.dram_tensor("w1", [CIN, H1], f32r, kind="ExternalInput")
    rs1_d = nc.dram_tensor("rs1w", [CIN, H1], f32r, kind="ExternalInput")
    w2_d = nc.dram_tensor("w2", [H1, H2], f32r, kind="ExternalInput")
    w3_d = nc.dram_tensor("w3", [H2, COUT], f32r, kind="ExternalInput")
    fcw_d = nc.dram_tensor("fcw", [COUT, FCO], f32r, kind="ExternalInput")
    sew1_d = nc.dram_tensor("sew1", [CIN, SEH], f32r, kind="ExternalInput")  # pre-scaled 1/21
    sew2_d = nc.dram_tensor("sew2", [SEH, CIN], f32r, kind="ExternalInput")
    abd_d = nc.dram_tensor("abd", [SUB_G * N, SUB_G * N], bf16, kind="ExternalInput")
    pk_d = {}
    for nm, nch in [("b1p", 8), ("bn1gp", 8), ("bn1bp", 8), ("rs1bp", 8),
                    ("b2p", 8), ("bn2gp", 8), ("bn2bp", 8),
                    ("b3p", 4), ("bn3gp", 4), ("bn3bp", 4), ("fcbp", 8)]:
        pk_d[nm] = nc.dram_tensor(nm, [128, nch], f32, kind="ExternalInput")
    out_d = nc.dram_tensor("out", [FCO, gpc], f32, kind="ExternalOutput")

    with tile.TileContext(nc) as tc:
        with tc.tile_pool(name="persist", bufs=1) as pp, \
             tc.tile_pool(name="scratch", bufs=2) as scp, \
             tc.tile_pool(name="dram", bufs=1, space="DRAM") as dmp, \
             tc.tile_pool(name="ps_hm", bufs=2, space="PSUM") as ps_hm, \
             tc.tile_pool(name="ps_small", bufs=3, space="PSUM") as ps_s:

            # DRAM spill buffers (pool tiles so Tile tracks deps), all f32
            z1_t = dmp.tile([H1, ROWS], f16, tag="z1")
            r1_t = dmp.tile([H1, ROWS], f16, tag="r1")
            u_t = dmp.tile([H1, ROWS], f16, tag="usp")
            z2_t = dmp.tile([H2, ROWS], f16, tag="z2")
            stb_in = {l: dmp.tile([128, 2 * nch], f32, tag=f"stbi{l}", name=f"stbi{l}")
                      for l, nch in [(1, 8), (2, 8), (3, 4)]}
            stb_out = {l: dmp.tile([128, 2 * nch], f32, tag=f"stbo{l}", name=f"stbo{l}")
                       for l, nch in [(1, 8), (2, 8), (3, 4)]}

            # packed per-channel params
            pk = {}
            for nm, d in pk_d.items():
                t = pp.tile(list(d.shape), f32, tag=nm, name=nm)
                nc.sync.dma_start(t[:], d.ap()[:])
                pk[nm] = t
            abd_sb = pp.tile([SUB_G * N, SUB_G * N], bf16, tag="abd")
            nc.sync.dma_start(abd_sb[:], abd_d.ap()[:])

            # bn stat slots + pooled accumulators + affine params
            bnsl = {1: [pp.tile([128, n_st * 6], f32, tag=f"bnsl1_{c}", name=f"bnsl1_{c}") for c in range(8)],
                    2: [pp.tile([128, n_st * 6], f32, tag=f"bnsl2_{c}", name=f"bnsl2_{c}") for c in range(8)],
                    3: [pp.tile([128, n_st * 6], f32, tag=f"bnsl3_{c}", name=f"bnsl3_{c}") for c in range(4)]}
            pool_sb = [pp.tile([128, gpc], f32, tag=f"pool_{c}", name=f"pool_{c}") for c in range(4)]
            sv = {1: pp.tile([128, 8], f32, tag="s1v", name="s1v"),
                  2: pp.tile([128, 8], f32, tag="s2v", name="s2v"),
                  3: pp.tile([128, 4], f32, tag="s3v", name="s3v")}
            tv = {1: pp.tile([128, 8], f32, tag="t1v", name="t1v"),
                  2: pp.tile([128, 8], f32, tag="t2v", name="t2v"),
                  3: pp.tile([128, 4], f32, tag="t3v", name="t3v")}
            s3p = pp.tile([128, 4], f32, tag="s3p")   # s3 / 21 for pooled affine

            def finalize_stats(l, nch, gp, bp, extra_bias=None):
                """bn slots -> AllReduce -> affine params sv[l], tv[l]."""
                loc = scp.tile([128, 2 * nch], f32, tag=f"loc{l}", name=f"loc{l}")
                for c in range(nch):
                    ag = scp.tile([128, 2], f32, tag="bnag", name="bnag")
                    nc.vector.bn_aggr(ag[:], bnsl[l][c][:])
                    nc.vector.tensor_copy(loc[:, c:c + 1], ag[:, 0:1])
                    # Ez2 = mean*mean + var
                    nc.vector.scalar_tensor_tensor(
                        loc[:, nch + c:nch + c + 1], ag[:, 0:1], ag[:, 0:1],
                        ag[:, 1:2], op0=OP.mult, op1=OP.add)
                nc.sync.dma_start(stb_in[l][:], loc[:])
                nc.gpsimd.collective_compute(
                    "AllReduce", OP.add, replica_groups=rg,
                    ins=[stb_in[l][:].opt()], outs=[stb_out[l][:].opt()])
                red = scp.tile([128, 2 * nch], f32, tag=f"red{l}", name=f"red{l}")
                nc.sync.dma_start(red[:], stb_out[l][:])
                mg = scp.tile([128, nch], f32, tag=f"mg{l}", name=f"mg{l}")
                e2 = scp.tile([128, nch], f32, tag=f"e2{l}", name=f"e2{l}")
                nc.vector.tensor_scalar_mul(mg[:], red[:, :nch], 1.0 / n_cores)
                nc.vector.tensor_scalar_mul(e2[:], red[:, nch:], 1.0 / n_cores)
                var = scp.tile([128, nch], f32, tag=f"var{l}", name=f"var{l}")
                nc.vector.tensor_tensor(var[:], mg[:], mg[:], op=OP.mult)
                nc.vector.tensor_tensor(var[:], e2[:], var[:], op=OP.subtract)
                nc.vector.tensor_scalar_add(var[:], var[:], EPS)
                sq = scp.tile([128, nch], f32, tag=f"sq{l}", name=f"sq{l}")
                nc.scalar.activation(sq[:], var[:], AF.Sqrt)
                y0 = scp.tile([128, nch], f32, tag=f"y0{l}", name=f"y0{l}")
                nc.vector.reciprocal(y0[:], sq[:])
                # one Newton step: y1 = y0 * (1.5 - 0.5 * var * y0^2)
                yy = scp.tile([128, nch], f32, tag=f"yy{l}", name=f"yy{l}")
                nc.vector.tensor_tensor(yy[:], y0[:], y0[:], op=OP.mult)
                nc.vector.tensor_tensor(yy[:], var[:], yy[:], op=OP.mult)
                nc.vector.tensor_scalar(yy[:], yy[:], -0.5, 1.5,
                                        op0=OP.mult, op1=OP.add)
                nc.vector.tensor_tensor(y0[:], y0[:], yy[:], op=OP.mult)
                nc.vector.tensor_tensor(sv[l][:, :nch], gp[:], y0[:], op=OP.mult)
                ms = scp.tile([128, nch], f32, tag=f"ms{l}", name=f"ms{l}")
                nc.vector.tensor_tensor(ms[:], mg[:], sv[l][:, :nch], op=OP.mult)
                nc.vector.tensor_tensor(tv[l][:, :nch], bp[:], ms[:], op=OP.subtract)
                if extra_bias is not None:
                    nc.vector.tensor_tensor(tv[l][:, :nch], tv[l][:, :nch],
                                            extra_bias[:], op=OP.add)

            # ================= Phase A: SE gate + layer 1 + residual ====
            with tc.tile_pool(name="wA", bufs=1) as wp, \
                 tc.tile_pool(name="actA", bufs=2) as ap_:
                w1_sb = []
                rs1_sb = []
                sew1_sb = []
                for k in range(4):
                    t = wp.tile([128, H1], f32r, tag=f"w1_{k}", name=f"w1sb_{k}")
                    nc.sync.dma_start(t[:], w1_d.ap()[k * 128:(k + 1) * 128, :])
                    w1_sb.append(t)
                    t = wp.tile([128, H1], f32r, tag=f"rs1_{k}", name=f"rs1sb_{k}")
                    nc.sync.dma_start(t[:], rs1_d.ap()[k * 128:(k + 1) * 128, :])
                    rs1_sb.append(t)
                    t = wp.tile([128, SEH], f32r, tag=f"sew1_{k}", name=f"sew1sb_{k}")
                    nc.sync.dma_start(t[:], sew1_d.ap()[k * 128:(k + 1) * 128, :])
                    sew1_sb.append(t)
                sew2_sb = wp.tile([128, CIN], f32r, tag="sew2")
                nc.sync.dma_start(sew2_sb[:], sew2_d.ap()[:])

                for sti, (g0, gc) in enumerate(sts):
                    rows = gc * N
                    c0 = g0 * N
                    x_sb = []
                    for k in range(4):
                        t = ap_.tile([128, STR], f32r, tag=f"x_{k}", name=f"x_{k}")
                        nc.sync.dma_start(t[:, :rows],
                                          xT.ap()[k * 128:(k + 1) * 128, c0:c0 + rows])
                        x_sb.append(t)
                    # SE: node-sum (1/21 folded into sew1) -> 2 tiny mms
                    xm = []
                    for k in range(4):
                        t = ap_.tile([128, ST_G], f32r, tag=f"xm_{k}", name=f"xm_{k}")
                        with nc.allow_low_precision(reason="fp32r rounding for SE matmul"):
                            nc.vector.tensor_reduce(
                                t[:, :gc],
                                x_sb[k][:, :rows].rearrange("p (g n) -> p g n", n=N),
                                axis=AX, op=OP.add)
                        xm.append(t)
                    y1ps = ps_s.tile([128, STR], f32, tag="small", name="y1ps")
                    for k in range(4):
                        nc.tensor.matmul(y1ps[:, :gc], lhsT=sew1_sb[k][:],
                                         rhs=xm[k][:, :gc],
                                         start=(k == 0), stop=(k == 3))
                    y1_sb = ap_.tile([128, ST_G], f32r, tag="y1", name="y1")
                    nc.scalar.activation(y1_sb[:, :gc], y1ps[:, :gc], AF.Relu)
                    y_sb = []
                    for m in range(4):
                        y2ps = ps_s.tile([128, STR], f32, tag="small", name="y2ps")
                        nc.tensor.matmul(y2ps[:, :gc],
                                         lhsT=sew2_sb[:, m * 128:(m + 1) * 128],
                                         rhs=y1_sb[:, :gc], start=True, stop=True)
                        t = ap_.tile([128, ST_G], f32, tag=f"y_{m}", name=f"y_{m}")
                        nc.scalar.activation(t[:, :gc], y2ps[:, :gc], AF.Sigmoid)
                        y_sb.append(t)
                    # gate
                    xg = []
                    for k in range(4):
                        t = ap_.tile([128, STR], f32r, tag=f"xg_{k}", name=f"xg_{k}")
                        nc.vector.tensor_tensor(
                            t[:, :rows].rearrange("p (g n) -> p g n", n=N),
                            x_sb[k][:, :rows].rearrange("p (g n) -> p g n", n=N),
                            y_sb[k][:, :gc].broadcast_to([128, gc, N]),
                            op=OP.mult)
                        xg.append(t)
                    # layer-1 matmul + message passing
                    z1sb = [ap_.tile([128, STR], f16, tag=f"z1sb_{c}", name=f"z1sb_{c}")
                            for c in range(8)]
                    for (r0, rr) in _subtiles(gc):
                        hmps = ps_hm.tile([128, H1], f32, tag="hm", name="hmps")
                        for k in range(4):
                            for n2 in range(2):
                                nc.tensor.matmul(
                                    hmps[:rr, n2 * 512:(n2 + 1) * 512],
                                    lhsT=xg[k][:, r0:r0 + rr],
                                    rhs=w1_sb[k][:, n2 * 512:(n2 + 1) * 512],
                                    start=(k == 0), stop=(k == 3))
                        hm_sb = ap_.tile([128, H1], bf16, tag="hm_sb", name="hm_sb")
                        nc.vector.tensor_copy(hm_sb[:rr, :], hmps[:rr, :])
                        for c in range(8):
                            aggps = ps_s.tile([128, STR], f32, tag="small", name="aggps")
                            nc.tensor.matmul(aggps[:, :rr],
                                             lhsT=hm_sb[:rr, c * 128:(c + 1) * 128],
                                             rhs=abd_sb[:rr, :rr],
                                             start=True, stop=True)
                            nc.scalar.activation(z1sb[c][:, r0:r0 + rr],
                                                 aggps[:, :rr], AF.Identity,
                                                 bias=pk["b1p"][:, c:c + 1])
                    for c in range(8):
                        nc.sync.dma_start(z1_t[c * 128:(c + 1) * 128, c0:c0 + rows],
                                          z1sb[c][:, :rows])
                        nc.vector.bn_stats(bnsl[1][c][:, sti * 6:(sti + 1) * 6],
                                           z1sb[c][:, :rows])
                    # residual r1 = xg @ rs1_w (channel-major-out)
                    for c in range(8):
                        r1ps = ps_s.tile([128, STR], f32, tag="small", name="r1ps")
                        for k in range(4):
                            nc.tensor.matmul(r1ps[:, :rows],
                                             lhsT=rs1_sb[k][:, c * 128:(c + 1) * 128],
                                             rhs=xg[k][:, :rows],
                                             start=(k == 0), stop=(k == 3))
                        r1sb = ap_.tile([128, STR], f16, tag="r1sb", name="r1sb")
                        nc.vector.tensor_copy(r1sb[:, :rows], r1ps[:, :rows])
                        nc.sync.dma_start(r1_t[c * 128:(c + 1) * 128, c0:c0 + rows],
                                          r1sb[:, :rows])

                finalize_stats(1, 8, pk["bn1gp"], pk["bn1bp"], pk["rs1bp"])

            # ================= Phase B: junction 1 + layer 2 ============
            with tc.tile_pool(name="wB", bufs=1) as wp, \
                 tc.tile_pool(name="actB", bufs=2) as ap_:
                w2_sb = []
                for k in range(8):
                    t = wp.tile([128, H2], f32r, tag=f"w2_{k}", name=f"w2sb_{k}")
                    nc.sync.dma_start(t[:], w2_d.ap()[k * 128:(k + 1) * 128, :])
                    w2_sb.append(t)
                for sti, (g0, gc) in enumerate(sts):
                    rows = gc * N
                    c0 = g0 * N
                    h1 = []
                    for c in range(8):
                        z1r = ap_.tile([128, STR], f16, tag=f"z1r_{c}", name=f"z1r_{c}")
                        nc.sync.dma_start(z1r[:, :rows],
                                          z1_t[c * 128:(c + 1) * 128, c0:c0 + rows])
                        r1r = ap_.tile([128, STR], f16, tag=f"r1r_{c}", name=f"r1r_{c}")
                        nc.sync.dma_start(r1r[:, :rows],
                                          r1_t[c * 128:(c + 1) * 128, c0:c0 + rows])
                        ut = ap_.tile([128, STR], f16, tag=f"ut_{c}", name=f"ut_{c}")
                        nc.vector.scalar_tensor_tensor(
                            ut[:, :rows], z1r[:, :rows], sv[1][:, c:c + 1],
                            r1r[:, :rows], op0=OP.mult, op1=OP.add)
                        nc.sync.dma_start(u_t[c * 128:(c + 1) * 128, c0:c0 + rows],
                                          ut[:, :rows])
                        t = ap_.tile([128, STR], f32r, tag=f"h1_{c}", name=f"h1_{c}")
                        nc.scalar.activation(t[:, :rows], ut[:, :rows], AF.Relu,
                                             bias=tv[1][:, c:c + 1])
                        h1.append(t)
                    z2sb = [ap_.tile([128, STR], f16, tag=f"z2sb_{c}", name=f"z2sb_{c}")
                            for c in range(8)]
                    for (r0, rr) in _subtiles(gc):
                        hmps = ps_hm.tile([128, H2], f32, tag="hm", name="hmps2")
                        for k in range(8):
                            for n2 in range(2):
                                nc.tensor.matmul(
                                    hmps[:rr, n2 * 512:(n2 + 1) * 512],
                                    lhsT=h1[k][:, r0:r0 + rr],
                                    rhs=w2_sb[k][:, n2 * 512:(n2 + 1) * 512],
                                    start=(k == 0), stop=(k == 7))
                        hm_sb = ap_.tile([128, H2], bf16, tag="hm_sb", name="hm_sb2")
                        nc.vector.tensor_copy(hm_sb[:rr, :], hmps[:rr, :])
                        for c in range(8):
                            aggps = ps_s.tile([128, STR], f32, tag="small", name="aggps2")
                            nc.tensor.matmul(aggps[:, :rr],
                                             lhsT=hm_sb[:rr, c * 128:(c + 1) * 128],
                                             rhs=abd_sb[:rr, :rr],
                                             start=True, stop=True)
                            nc.scalar.activation(z2sb[c][:, r0:r0 + rr],
                                                 aggps[:, :rr], AF.Identity,
                                                 bias=pk["b2p"][:, c:c + 1])
                    for c in range(8):
                        nc.sync.dma_start(z2_t[c * 128:(c + 1) * 128, c0:c0 + rows],
                                          z2sb[c][:, :rows])
                        nc.vector.bn_stats(bnsl[2][c][:, sti * 6:(sti + 1) * 6],
                                           z2sb[c][:, :rows])
                finalize_stats(2, 8, pk["bn2gp"], pk["bn2bp"])

            # ================= Phase C: junction 2 + layer 3 + pool =====
            with tc.tile_pool(name="wC", bufs=1) as wp, \
                 tc.tile_pool(name="actC", bufs=2) as ap_:
                w3_sb = []
                for k in range(8):
                    t = wp.tile([128, COUT], f32r, tag=f"w3_{k}", name=f"w3sb_{k}")
                    nc.sync.dma_start(t[:], w3_d.ap()[k * 128:(k + 1) * 128, :])
                    w3_sb.append(t)
                for sti, (g0, gc) in enumerate(sts):
                    rows = gc * N
                    c0 = g0 * N
                    h2 = []
                    for c in range(8):
                        z2r = ap_.tile([128, STR], f16, tag=f"z2r_{c}", name=f"z2r_{c}")
                        nc.sync.dma_start(z2r[:, :rows],
                                          z2_t[c * 128:(c + 1) * 128, c0:c0 + rows])
                        ur = ap_.tile([128, STR], f16, tag=f"ur_{c}", name=f"ur_{c}")
                        nc.sync.dma_start(ur[:, :rows],
                                          u_t[c * 128:(c + 1) * 128, c0:c0 + rows])
                        h1c = ap_.tile([128, STR], f16, tag=f"h1c_{c}", name=f"h1c_{c}")
                        nc.scalar.activation(h1c[:, :rows], ur[:, :rows], AF.Relu,
                                             bias=tv[1][:, c:c + 1])
                        bt = ap_.tile([128, STR], f16, tag=f"bt_{c}", name=f"bt_{c}")
                        nc.vector.scalar_tensor_tensor(
                            bt[:, :rows], z2r[:, :rows], sv[2][:, c:c + 1],
                            h1c[:, :rows], op0=OP.mult, op1=OP.add)
                        t = ap_.tile([128, STR], f32r, tag=f"h2_{c}", name=f"h2_{c}")
                        nc.scalar.activation(t[:, :rows], bt[:, :rows], AF.Relu,
                                             bias=tv[2][:, c:c + 1])
                        h2.append(t)
                    z3sb = [ap_.tile([128, STR], f32, tag=f"z3sb_{c}", name=f"z3sb_{c}")
                            for c in range(4)]
                    for (r0, rr) in _subtiles(gc):
                        hmps = ps_hm.tile([128, H2], f32, tag="hm", name="hmps3")
                        for k in range(8):
                            nc.tensor.matmul(hmps[:rr, :COUT],
                                             lhsT=h2[k][:, r0:r0 + rr],
                                             rhs=w3_sb[k][:],
                                             start=(k == 0), stop=(k == 7))
                        hm_sb = ap_.tile([128, H2], bf16, tag="hm_sb", name="hm_sb3")
                        nc.vector.tensor_copy(hm_sb[:rr, :COUT], hmps[:rr, :COUT])
                        for c in range(4):
                            aggps = ps_s.tile([128, STR], f32, tag="small", name="aggps3")
                            nc.tensor.matmul(aggps[:, :rr],
                                             lhsT=hm_sb[:rr, c * 128:(c + 1) * 128],
                                             rhs=abd_sb[:rr, :rr],
                                             start=True, stop=True)
                            nc.scalar.activation(z3sb[c][:, r0:r0 + rr],
                                                 aggps[:, :rr], AF.Identity,
                                                 bias=pk["b3p"][:, c:c + 1])
                    for c in range(4):
                        nc.vector.bn_stats(bnsl[3][c][:, sti * 6:(sti + 1) * 6],
                                           z3sb[c][:, :rows])
                        nc.vector.tensor_reduce(
                            pool_sb[c][:, g0:g0 + gc],
                            z3sb[c][:, :rows].rearrange("p (g n) -> p g n", n=N),
                            axis=AX, op=OP.add)
                finalize_stats(3, 4, pk["bn3gp"], pk["bn3bp"])
                nc.vector.tensor_scalar_mul(s3p[:], sv[3][:], 1.0 / N)

            # ================= Phase D: pooled affine + FC ==============
            with tc.tile_pool(name="wD", bufs=1) as wp, \
                 tc.tile_pool(name="actD", bufs=2) as ap_:
                fcw_sb = []
                for k in range(4):
                    t = wp.tile([128, FCO], f32r, tag=f"fcw_{k}", name=f"fcwsb_{k}")
                    nc.sync.dma_start(t[:], fcw_d.ap()[k * 128:(k + 1) * 128, :])
                    fcw_sb.append(t)
                pbn = []
                for c in range(4):
                    t = ap_.tile([128, gpc], f32r, tag=f"pbn_{c}", name=f"pbn_{c}")
                    nc.vector.tensor_scalar(t[:], pool_sb[c][:],
                                            s3p[:, c:c + 1], tv[3][:, c:c + 1],
                                            op0=OP.mult, op1=OP.add)
                    pbn.append(t)
                for m in range(8):
                    fcps = ps_s.tile([128, max(gpc, STR)], f32, tag="small", name="fcps")
                    for k in range(4):
                        nc.tensor.matmul(fcps[:, :gpc],
                                         lhsT=fcw_sb[k][:, m * 128:(m + 1) * 128],
                                         rhs=pbn[k][:],
                                         start=(k == 0), stop=(k == 3))
                    osb = ap_.tile([128, gpc], f32, tag="osb", name="osb")
                    nc.scalar.activation(osb[:], fcps[:, :gpc], AF.Identity,
                                         bias=pk["fcbp"][:, m:m + 1])
                    nc.sync.dma_start(out_d.ap()[m * 128:(m + 1) * 128, :], osb[:])

    nc.compile()
    return nc


def host_prep(inputs, gpc=GPC, n_cores=NCORES):
    """Build per-core in_maps from the full problem inputs."""
    g = lambda k: np.asarray(inputs[k], np.float32)
    A = _host_adjacency(inputs["nodevec1"], inputs["nodevec2"])
    R = SUB_G * N
    abd = np.zeros((R, R), np.float32)
    for b in range(SUB_G):
        abd[b * N:(b + 1) * N, b * N:(b + 1) * N] = A.T   # rhs[j, i] = A[i, j]
    abd = abd.astype(ml_dtypes.bfloat16)

    shared = {
        "w1": g("W1"), "rs1w": g("rs1_w"), "w2": g("W2"), "w3": g("W3"),
        "fcw": g("fc_w"),
        "sew1": (g("se_w1") / np.float32(N)).astype(np.float32),
        "sew2": g("se_w2"), "abd": abd,
        "b1p": _pack_ch(g("b1"), 8), "bn1gp": _pack_ch(g("bn1_g"), 8),
        "bn1bp": _pack_ch(g("bn1_b"), 8), "rs1bp": _pack_ch(g("rs1_b"), 8),
        "b2p": _pack_ch(g("b2"), 8), "bn2gp": _pack_ch(g("bn2_g"), 8),
        "bn2bp": _pack_ch(g("bn2_b"), 8),
        "b3p": _pack_ch(g("b3"), 4), "bn3gp": _pack_ch(g("bn3_g"), 4),
        "bn3bp": _pack_ch(g("bn3_b"), 4), "fcbp": _pack_ch(g("fc_b"), 8),
    }
    shared = {k: np.ascontiguousarray(v) for k, v in shared.items()}
    x = g("x")
    rows = gpc * N
    in_maps = []
    for i in range(n_cores):
        m = dict(shared)
        m["xT"] = np.ascontiguousarray(x[i * rows:(i + 1) * rows, :].T)
        in_maps.append(m)
    return in_maps


_cache = {}


def run(inputs, trace=False, trace_cores=None):
    from concourse.bass_utils import run_bass_kernel_spmd
    key = (GPC, NCORES)
    if key not in _cache:
        _cache[key] = build_nc(GPC, NCORES)
    nc = _cache[key]
    in_maps = host_prep(inputs, GPC, NCORES)
    res = run_bass_kernel_spmd(nc, in_maps, core_ids=list(range(NCORES)),
                               trace=trace, trace_cores=trace_cores)
    shards = [np.asarray(res.results[i]["out"]) for i in range(NCORES)]
    out = np.concatenate([s.T for s in shards], axis=0).astype(np.float32)
    return out, res


def kernel(**inputs) -> np.ndarray:
    out, _ = run(inputs, trace=False)
    return out


# revision 6
# speedup vs baseline: 1.1472x; 1.0550x over previous
"""AdaptiveGCN Trainium2 kernel — 8-core data-parallel over the graph/batch dim.

Layout strategy (per core, channel-major activations):
  - activations live as [C, rows] (channel on partitions), weights as [cin, cout].
  - channel matmul uses the row-major-out form: out[rows, cout] = lhsT(act).T @ rhs(W)
  - message passing uses the channel-major-out form:
      agg[c, i] = lhsT(hm[rows_j, c]).T @ rhs(A_blockdiag[rows_j, rows_i])
    so the layer output lands back in channel-major with zero transposes.
  - channel matmuls run as float32r (full PE rate at free-dim >= 256),
    message-passing matmuls run bf16 (tiny free dim).
  - BatchNorm is training-mode over ALL rows: local Welford stats via
    bn_stats/bn_aggr, merged across the 8 cores with one small AllReduce
    per layer. Activations round-trip HBM (f32) across the BN barriers.
"""

import numpy as np
import ml_dtypes

# problem dims (hardcoded per task spec)
B = 4096
N = 21
CIN = 512
H1 = 1024
H2 = 1024
COUT = 512
FCO = 1024
SEH = 128
RANK = 10
EPS = 1e-5
THRESH = 0.1
NCORES = 8
GPC = B // NCORES            # graphs per core
ST_G = 18                    # graphs per supertile (378 rows)
SUB_G = 6                    # graphs per matmul row-tile (126 rows)


def _host_adjacency(nodevec1, nodevec2):
    """Replicate the reference adjacency computation in f32 numpy."""
    nv1 = np.asarray(nodevec1, np.float32)
    nv2 = np.asarray(nodevec2, np.float32)
    logits = (nv1 @ nv2).astype(np.float32)
    adp = (1.0 / (1.0 + np.exp(-logits, dtype=np.float32))).astype(np.float32)
    at = (adp + np.eye(N, dtype=np.float32)).astype(np.float32)
    deg = at.sum(1, dtype=np.float32)
    dinv = np.where(deg > 0, deg.astype(np.float32) ** -0.5, 0.0).astype(np.float32)
    A = (dinv[:, None] * at * dinv[None, :]).astype(np.float32)
    return np.where(A > THRESH, A, 0.0).astype(np.float32)


def _pack_ch(vec, nch):
    """[nch*128] channel vector -> [128, nch] (partition = channel % 128)."""
    v = np.asarray(vec, np.float32).reshape(nch, 128)
    return np.ascontiguousarray(v.T)


def _supertiles(gpc):
    out, g = [], 0
    while g < gpc:
        gc = min(ST_G, gpc - g)
        out.append((g, gc))
        g += gc
    return out


def _subtiles(gc):
    out, r, rows = [], 0, gc * N
    while r < rows:
        rr = min(SUB_G * N, rows - r)
        rpad = 128 if r + 128 <= rows else rr   # K=128-padded reads for FWL
        out.append((r, rr, rpad))
        out[-1] = (r, rr, rpad)
        r += rr
    return out


def build_nc(gpc=GPC, n_cores=NCORES):
    import concourse.bass as bass
    import concourse.bacc as bacc
    import concourse.mybir as mybir
    import concourse.tile as tile

    f32 = mybir.dt.float32
    f32r = mybir.dt.float32r
    bf16 = mybir.dt.bfloat16
    f16 = mybir.dt.float16
    AF = mybir.ActivationFunctionType
    OP = mybir.AluOpType
    AX = mybir.AxisListType.X

    ROWS = gpc * N
    STR = ST_G * N               # supertile rows (378)
    sts = _supertiles(gpc)
    n_st = len(sts)
    rg = [list(range(n_cores))]
    
    nc = bacc.Bacc("TRN2", target_bir_lowering=False, debug=False,
                   num_devices=n_cores)

    # ---- external I/O -------------------------------------------------
    xT = nc.dram_tensor("xT", [CIN, ROWS], f16, kind="ExternalInput")
    w1_d = nc.dram_tensor("w1", [CIN, H1], f16, kind="ExternalInput")
    rs1_d = nc.dram_tensor("rs1w", [CIN, H1], f16, kind="ExternalInput")
    w2_d = nc.dram_tensor("w2", [H1, H2], f16, kind="ExternalInput")
    w3_d = nc.dram_tensor("w3", [H2, COUT], f16, kind="ExternalInput")
    fcw_d = nc.dram_tensor("fcw", [COUT, FCO], f16, kind="ExternalInput")
    sew1_d = nc.dram_tensor("sew1", [CIN, SEH], f16, kind="ExternalInput")  # pre-scaled 1/21
    sew2_d = nc.dram_tensor("sew2", [SEH, CIN], f16, kind="ExternalInput")
    abd_d = nc.dram_tensor("abd", [128, SUB_G * N], f16, kind="ExternalInput")
    pk_d = {}
    for nm, nch in [("b1p", 8), ("bn1gp", 8), ("bn1bp", 8), ("rs1bp", 8),
                    ("b2p", 8), ("bn2gp", 8), ("bn2bp", 8),
                    ("b3p", 4), ("bn3gp", 4), ("bn3bp", 4), ("fcbp", 8)]:
        pk_d[nm] = nc.dram_tensor(nm, [128, nch], f32, kind="ExternalInput")
    out_d = nc.dram_tensor("out", [FCO, gpc], f32, kind="ExternalOutput")

    with tile.TileContext(nc) as tc:
        with tc.tile_pool(name="persist", bufs=1) as pp, \
             tc.tile_pool(name="scratch", bufs=2) as scp, \
             tc.tile_pool(name="dram", bufs=1, space="DRAM") as dmp, \
             tc.tile_pool(name="ps_hm", bufs=2, space="PSUM") as ps_hm, \
             tc.tile_pool(name="ps_small", bufs=3, space="PSUM") as ps_s:

            # DRAM spill buffers (pool tiles so Tile tracks deps), all f32
            z1_t = dmp.tile([H1, ROWS], f16, tag="z1")
            r1_t = dmp.tile([H1, ROWS], f16, tag="r1")
            u_t = dmp.tile([H1, ROWS], f16, tag="usp")
            z2_t = dmp.tile([H2, ROWS], f16, tag="z2")
            stb_in = {l: dmp.tile([128, 2 * nch], f32, tag=f"stbi{l}", name=f"stbi{l}")
                      for l, nch in [(1, 8), (2, 8), (3, 4)]}
            stb_out = {l: dmp.tile([128, 2 * nch], f32, tag=f"stbo{l}", name=f"stbo{l}")
                       for l, nch in [(1, 8), (2, 8), (3, 4)]}

            # packed per-channel params
            pk = {}
            for nm, d in pk_d.items():
                t = pp.tile(list(d.shape), f32, tag=nm, name=nm)
                nc.sync.dma_start(t[:], d.ap()[:])
                pk[nm] = t
            abd_sb = pp.tile([128, SUB_G * N], f16, tag="abd")
            nc.sync.dma_start(abd_sb[:], abd_d.ap()[:])

            # bn stat slots + pooled accumulators + affine params
            bnsl = {1: [pp.tile([128, n_st * 6], f32, tag=f"bnsl1_{c}", name=f"bnsl1_{c}") for c in range(8)],
                    2: [pp.tile([128, n_st * 6], f32, tag=f"bnsl2_{c}", name=f"bnsl2_{c}") for c in range(8)],
                    3: [pp.tile([128, n_st * 6], f32, tag=f"bnsl3_{c}", name=f"bnsl3_{c}") for c in range(4)]}
            pool_sb = [pp.tile([128, gpc], f32, tag=f"pool_{c}", name=f"pool_{c}") for c in range(4)]
            sv = {1: pp.tile([128, 8], f32, tag="s1v", name="s1v"),
                  2: pp.tile([128, 8], f32, tag="s2v", name="s2v"),
                  3: pp.tile([128, 4], f32, tag="s3v", name="s3v")}
            tv = {1: pp.tile([128, 8], f32, tag="t1v", name="t1v"),
                  2: pp.tile([128, 8], f32, tag="t2v", name="t2v"),
                  3: pp.tile([128, 4], f32, tag="t3v", name="t3v")}
            s3p = pp.tile([128, 4], f32, tag="s3p")   # s3 / 21 for pooled affine

            def finalize_stats(l, nch, gp, bp, extra_bias=None):
                """bn slots -> AllReduce -> affine params sv[l], tv[l]."""
                loc = scp.tile([128, 2 * nch], f32, tag=f"loc{l}", name=f"loc{l}")
                for c in range(nch):
                    ag = scp.tile([128, 2], f32, tag="bnag", name="bnag")
                    nc.vector.bn_aggr(ag[:], bnsl[l][c][:])
                    nc.vector.tensor_copy(loc[:, c:c + 1], ag[:, 0:1])
                    # Ez2 = mean*mean + var
                    nc.vector.scalar_tensor_tensor(
                        loc[:, nch + c:nch + c + 1], ag[:, 0:1], ag[:, 0:1],
                        ag[:, 1:2], op0=OP.mult, op1=OP.add)
                nc.sync.dma_start(stb_in[l][:], loc[:])
                nc.gpsimd.collective_compute(
                    "AllReduce", OP.add, replica_groups=rg,
                    ins=[stb_in[l][:].opt()], outs=[stb_out[l][:].opt()])
                red = scp.tile([128, 2 * nch], f32, tag=f"red{l}", name=f"red{l}")
                nc.sync.dma_start(red[:], stb_out[l][:])
                mg = scp.tile([128, nch], f32, tag=f"mg{l}", name=f"mg{l}")
                e2 = scp.tile([128, nch], f32, tag=f"e2{l}", name=f"e2{l}")
                nc.vector.tensor_scalar_mul(mg[:], red[:, :nch], 1.0 / n_cores)
                nc.vector.tensor_scalar_mul(e2[:], red[:, nch:], 1.0 / n_cores)
                var = scp.tile([128, nch], f32, tag=f"var{l}", name=f"var{l}")
                nc.vector.tensor_tensor(var[:], mg[:], mg[:], op=OP.mult)
                nc.vector.tensor_tensor(var[:], e2[:], var[:], op=OP.subtract)
                nc.vector.tensor_scalar_add(var[:], var[:], EPS)
                sq = scp.tile([128, nch], f32, tag=f"sq{l}", name=f"sq{l}")
                nc.scalar.activation(sq[:], var[:], AF.Sqrt)
                y0 = scp.tile([128, nch], f32, tag=f"y0{l}", name=f"y0{l}")
                nc.vector.reciprocal(y0[:], sq[:])
                # one Newton step: y1 = y0 * (1.5 - 0.5 * var * y0^2)
                yy = scp.tile([128, nch], f32, tag=f"yy{l}", name=f"yy{l}")
                nc.vector.tensor_tensor(yy[:], y0[:], y0[:], op=OP.mult)
                nc.vector.tensor_tensor(yy[:], var[:], yy[:], op=OP.mult)
                nc.vector.tensor_scalar(yy[:], yy[:], -0.5, 1.5,
                                        op0=OP.mult, op1=OP.add)
                nc.vector.tensor_tensor(y0[:], y0[:], yy[:], op=OP.mult)
                nc.vector.tensor_tensor(sv[l][:, :nch], gp[:], y0[:], op=OP.mult)
                ms = scp.tile([128, nch], f32, tag=f"ms{l}", name=f"ms{l}")
                nc.vector.tensor_tensor(ms[:], mg[:], sv[l][:, :nch], op=OP.mult)
                nc.vector.tensor_tensor(tv[l][:, :nch], bp[:], ms[:], op=OP.subtract)
                if extra_bias is not None:
                    nc.vector.tensor_tensor(tv[l][:, :nch], tv[l][:, :nch],
                                            extra_bias[:], op=OP.add)

            # ================= Phase A: SE gate + layer 1 + residual ====
            with tc.tile_pool(name="wA", bufs=1) as wp, \
                 tc.tile_pool(name="actA", bufs=2) as ap_:
                w1_sb = []
                rs1_sb = []
                sew1_sb = []
                for k in range(4):
                    t = wp.tile([128, H1], f16, tag=f"w1_{k}", name=f"w1sb_{k}")
                    nc.sync.dma_start(t[:], w1_d.ap()[k * 128:(k + 1) * 128, :])
                    w1_sb.append(t)
                    t = wp.tile([128, H1], f16, tag=f"rs1_{k}", name=f"rs1sb_{k}")
                    nc.sync.dma_start(t[:], rs1_d.ap()[k * 128:(k + 1) * 128, :])
                    rs1_sb.append(t)
                    t = wp.tile([128, SEH], f16, tag=f"sew1_{k}", name=f"sew1sb_{k}")
                    nc.sync.dma_start(t[:], sew1_d.ap()[k * 128:(k + 1) * 128, :])
                    sew1_sb.append(t)
                sew2_sb = wp.tile([128, CIN], f16, tag="sew2")
                nc.sync.dma_start(sew2_sb[:], sew2_d.ap()[:])

                for sti, (g0, gc) in enumerate(sts):
                    rows = gc * N
                    c0 = g0 * N
                    x_sb = []
                    for k in range(4):
                        t = ap_.tile([128, STR], f16, tag=f"x_{k}", name=f"x_{k}")
                        nc.sync.dma_start(t[:, :rows],
                                          xT.ap()[k * 128:(k + 1) * 128, c0:c0 + rows])
                        x_sb.append(t)
                    # SE: node-sum (1/21 folded into sew1) -> 2 tiny mms
                    xm = []
                    for k in range(4):
                        t = ap_.tile([128, ST_G], f16, tag=f"xm_{k}", name=f"xm_{k}")
                        with nc.allow_low_precision(reason="fp32r rounding for SE matmul"):
                            nc.vector.tensor_reduce(
                                t[:, :gc],
                                x_sb[k][:, :rows].rearrange("p (g n) -> p g n", n=N),
                                axis=AX, op=OP.add)
                        xm.append(t)
                    y1ps = ps_s.tile([128, STR], f32, tag="small", name="y1ps")
                    for k in range(4):
                        nc.tensor.matmul(y1ps[:, :gc], lhsT=sew1_sb[k][:],
                                         rhs=xm[k][:, :gc],
                                         start=(k == 0), stop=(k == 3))
                    y1_sb = ap_.tile([128, ST_G], f16, tag="y1", name="y1")
                    nc.scalar.activation(y1_sb[:, :gc], y1ps[:, :gc], AF.Relu)
                    y_sb = []
                    for m in range(4):
                        y2ps = ps_s.tile([128, STR], f32, tag="small", name="y2ps")
                        nc.tensor.matmul(y2ps[:, :gc],
                                         lhsT=sew2_sb[:, m * 128:(m + 1) * 128],
                                         rhs=y1_sb[:, :gc], start=True, stop=True)
                        t = ap_.tile([128, ST_G], f32, tag=f"y_{m}", name=f"y_{m}")
                        nc.scalar.activation(t[:, :gc], y2ps[:, :gc], AF.Sigmoid)
                        y_sb.append(t)
                    # gate
                    xg = []
                    for k in range(4):
                        t = ap_.tile([128, STR], f16, tag=f"xg_{k}", name=f"xg_{k}")
                        nc.vector.tensor_tensor(
                            t[:, :rows].rearrange("p (g n) -> p g n", n=N),
                            x_sb[k][:, :rows].rearrange("p (g n) -> p g n", n=N),
                            y_sb[k][:, :gc].broadcast_to([128, gc, N]),
                            op=OP.mult)
                        xg.append(t)
                    # layer-1 matmul + message passing
                    z1sb = [ap_.tile([128, STR], f16, tag=f"z1sb_{c}", name=f"z1sb_{c}")
                            for c in range(8)]
                    for (r0, rr, rp) in _subtiles(gc):
                        hmps = ps_hm.tile([128, H1], f32, tag="hm", name="hmps")
                        for k in range(4):
                            for n2 in range(2):
                                nc.tensor.matmul(
                                    hmps[:rp, n2 * 512:(n2 + 1) * 512],
                                    lhsT=xg[k][:, r0:r0 + rp],
                                    rhs=w1_sb[k][:, n2 * 512:(n2 + 1) * 512],
                                    start=(k == 0), stop=(k == 3))
                        hm_sb = ap_.tile([128, H1], f16, tag="hm_sb", name="hm_sb")
                        nc.vector.tensor_copy(hm_sb[:rp, :], hmps[:rp, :])
                        for c in range(8):
                            aggps = ps_s.tile([128, STR], f32, tag="small", name="aggps")
                            nc.tensor.matmul(aggps[:, :rr],
                                             lhsT=hm_sb[:rp, c * 128:(c + 1) * 128],
                                             rhs=abd_sb[:rp, :rr],
                                             start=True, stop=True)
                            nc.scalar.activation(z1sb[c][:, r0:r0 + rr],
                                                 aggps[:, :rr], AF.Identity,
                                                 bias=pk["b1p"][:, c:c + 1])
                    for c in range(8):
                        nc.sync.dma_start(z1_t[c * 128:(c + 1) * 128, c0:c0 + rows],
                                          z1sb[c][:, :rows])
                        nc.vector.bn_stats(bnsl[1][c][:, sti * 6:(sti + 1) * 6],
                                           z1sb[c][:, :rows])
                    # residual r1 = xg @ rs1_w (channel-major-out)
                    for c in range(8):
                        r1ps = ps_s.tile([128, STR], f32, tag="small", name="r1ps")
                        for k in range(4):
                            nc.tensor.matmul(r1ps[:, :rows],
                                             lhsT=rs1_sb[k][:, c * 128:(c + 1) * 128],
                                             rhs=xg[k][:, :rows],
                                             start=(k == 0), stop=(k == 3))
                        r1sb = ap_.tile([128, STR], f16, tag="r1sb", name="r1sb")
                        nc.vector.tensor_copy(r1sb[:, :rows], r1ps[:, :rows])
                        nc.sync.dma_start(r1_t[c * 128:(c + 1) * 128, c0:c0 + rows],
                                          r1sb[:, :rows])

                finalize_stats(1, 8, pk["bn1gp"], pk["bn1bp"], pk["rs1bp"])

            # ================= Phase B: junction 1 + layer 2 ============
            with tc.tile_pool(name="wB", bufs=1) as wp, \
                 tc.tile_pool(name="actB", bufs=2) as ap_:
                w2_sb = []
                for k in range(8):
                    t = wp.tile([128, H2], f16, tag=f"w2_{k}", name=f"w2sb_{k}")
                    nc.sync.dma_start(t[:], w2_d.ap()[k * 128:(k + 1) * 128, :])
                    w2_sb.append(t)
                for sti, (g0, gc) in enumerate(sts):
                    rows = gc * N
                    c0 = g0 * N
                    h1 = []
                    for c in range(8):
                        z1r = ap_.tile([128, STR], f16, tag=f"z1r_{c}", name=f"z1r_{c}")
                        nc.sync.dma_start(z1r[:, :rows],
                                          z1_t[c * 128:(c + 1) * 128, c0:c0 + rows])
                        r1r = ap_.tile([128, STR], f16, tag=f"r1r_{c}", name=f"r1r_{c}")
                        nc.sync.dma_start(r1r[:, :rows],
                                          r1_t[c * 128:(c + 1) * 128, c0:c0 + rows])
                        ut = ap_.tile([128, STR], f16, tag=f"ut_{c}", name=f"ut_{c}")
                        nc.vector.scalar_tensor_tensor(
                            ut[:, :rows], z1r[:, :rows], sv[1][:, c:c + 1],
                            r1r[:, :rows], op0=OP.mult, op1=OP.add)
                        nc.sync.dma_start(u_t[c * 128:(c + 1) * 128, c0:c0 + rows],
                                          ut[:, :rows])
                        t = ap_.tile([128, STR], f16, tag=f"h1_{c}", name=f"h1_{c}")
                        nc.scalar.activation(t[:, :rows], ut[:, :rows], AF.Relu,
                                             bias=tv[1][:, c:c + 1])
                        h1.append(t)
                    z2sb = [ap_.tile([128, STR], f16, tag=f"z2sb_{c}", name=f"z2sb_{c}")
                            for c in range(8)]
                    for (r0, rr, rp) in _subtiles(gc):
                        hmps = ps_hm.tile([128, H2], f32, tag="hm", name="hmps2")
                        for k in range(8):
                            for n2 in range(2):
                                nc.tensor.matmul(
                                    hmps[:rp, n2 * 512:(n2 + 1) * 512],
                                    lhsT=h1[k][:, r0:r0 + rp],
                                    rhs=w2_sb[k][:, n2 * 512:(n2 + 1) * 512],
                                    start=(k == 0), stop=(k == 7))
                        hm_sb = ap_.tile([128, H2], f16, tag="hm_sb", name="hm_sb2")
                        nc.vector.tensor_copy(hm_sb[:rp, :], hmps[:rp, :])
                        for c in range(8):
                            aggps = ps_s.tile([128, STR], f32, tag="small", name="aggps2")
                            nc.tensor.matmul(aggps[:, :rr],
                                             lhsT=hm_sb[:rp, c * 128:(c + 1) * 128],
                                             rhs=abd_sb[:rp, :rr],
                                             start=True, stop=True)
                            nc.scalar.activation(z2sb[c][:, r0:r0 + rr],
                                                 aggps[:, :rr], AF.Identity,
                                                 bias=pk["b2p"][:, c:c + 1])
                    for c in range(8):
                        nc.sync.dma_start(z2_t[c * 128:(c + 1) * 128, c0:c0 + rows],
                                          z2sb[c][:, :rows])
                        nc.vector.bn_stats(bnsl[2][c][:, sti * 6:(sti + 1) * 6],
                                           z2sb[c][:, :rows])
                finalize_stats(2, 8, pk["bn2gp"], pk["bn2bp"])

            # ================= Phase C: junction 2 + layer 3 + pool =====
            with tc.tile_pool(name="wC", bufs=1) as wp, \
                 tc.tile_pool(name="actC", bufs=2) as ap_:
                w3_sb = []
                for k in range(8):
                    t = wp.tile([128, COUT], f16, tag=f"w3_{k}", name=f"w3sb_{k}")
                    nc.sync.dma_start(t[:], w3_d.ap()[k * 128:(k + 1) * 128, :])
                    w3_sb.append(t)
                for sti, (g0, gc) in enumerate(sts):
                    rows = gc * N
                    c0 = g0 * N
                    h2 = []
                    for c in range(8):
                        z2r = ap_.tile([128, STR], f16, tag=f"z2r_{c}", name=f"z2r_{c}")
                        nc.sync.dma_start(z2r[:, :rows],
                                          z2_t[c * 128:(c + 1) * 128, c0:c0 + rows])
                        ur = ap_.tile([128, STR], f16, tag=f"ur_{c}", name=f"ur_{c}")
                        nc.sync.dma_start(ur[:, :rows],
                                          u_t[c * 128:(c + 1) * 128, c0:c0 + rows])
                        h1c = ap_.tile([128, STR], f16, tag=f"h1c_{c}", name=f"h1c_{c}")
                        nc.scalar.activation(h1c[:, :rows], ur[:, :rows], AF.Relu,
                                             bias=tv[1][:, c:c + 1])
                        bt = ap_.tile([128, STR], f16, tag=f"bt_{c}", name=f"bt_{c}")
                        nc.vector.scalar_tensor_tensor(
                            bt[:, :rows], z2r[:, :rows], sv[2][:, c:c + 1],
                            h1c[:, :rows], op0=OP.mult, op1=OP.add)
                        t = ap_.tile([128, STR], f16, tag=f"h2_{c}", name=f"h2_{c}")
                        nc.scalar.activation(t[:, :rows], bt[:, :rows], AF.Relu,
                                             bias=tv[2][:, c:c + 1])
                        h2.append(t)
                    z3sb = [ap_.tile([128, STR], f32, tag=f"z3sb_{c}", name=f"z3sb_{c}")
                            for c in range(4)]
                    for (r0, rr, rp) in _subtiles(gc):
                        hmps = ps_hm.tile([128, H2], f32, tag="hm", name="hmps3")
                        for k in range(8):
                            nc.tensor.matmul(hmps[:rp, :COUT],
                                             lhsT=h2[k][:, r0:r0 + rp],
                                             rhs=w3_sb[k][:],
                                             start=(k == 0), stop=(k == 7))
                        hm_sb = ap_.tile([128, H2], f16, tag="hm_sb", name="hm_sb3")
                        nc.vector.tensor_copy(hm_sb[:rp, :COUT], hmps[:rp, :COUT])
                        for c in range(4):
                            aggps = ps_s.tile([128, STR], f32, tag="small", name="aggps3")
                            nc.tensor.matmul(aggps[:, :rr],
                                             lhsT=hm_sb[:rp, c * 128:(c + 1) * 128],
                                             rhs=abd_sb[:rp, :rr],
                                             start=True, stop=True)
                            nc.scalar.activation(z3sb[c][:, r0:r0 + rr],
                                                 aggps[:, :rr], AF.Identity,
                                                 bias=pk["b3p"][:, c:c + 1])
                    for c in range(4):
                        nc.vector.bn_stats(bnsl[3][c][:, sti * 6:(sti + 1) * 6],
                                           z3sb[c][:, :rows])
                        nc.vector.tensor_reduce(
                            pool_sb[c][:, g0:g0 + gc],
                            z3sb[c][:, :rows].rearrange("p (g n) -> p g n", n=N),
                            axis=AX, op=OP.add)
                finalize_stats(3, 4, pk["bn3gp"], pk["bn3bp"])
                nc.vector.tensor_scalar_mul(s3p[:], sv[3][:], 1.0 / N)

            # ================= Phase D: pooled affine + FC ==============
            with tc.tile_pool(name="wD", bufs=1) as wp, \
                 tc.tile_pool(name="actD", bufs=2) as ap_:
                fcw_sb = []
                for k in range(4):
                    t = wp.tile([128, FCO], f16, tag=f"fcw_{k}", name=f"fcwsb_{k}")
                    nc.sync.dma_start(t[:], fcw_d.ap()[k * 128:(k + 1) * 128, :])
                    fcw_sb.append(t)
                pbn = []
                for c in range(4):
                    t = ap_.tile([128, gpc], f16, tag=f"pbn_{c}", name=f"pbn_{c}")
                    nc.vector.tensor_scalar(t[:], pool_sb[c][:],
                                            s3p[:, c:c + 1], tv[3][:, c:c + 1],
                                            op0=OP.mult, op1=OP.add)
                    pbn.append(t)
                for m in range(8):
                    fcps = ps_s.tile([128, max(gpc, STR)], f32, tag="small", name="fcps")
                    for k in range(4):
                        nc.tensor.matmul(fcps[:, :gpc],
                                         lhsT=fcw_sb[k][:, m * 128:(m + 1) * 128],
                                         rhs=pbn[k][:],
                                         start=(k == 0), stop=(k == 3))
                    osb = ap_.tile([128, gpc], f32, tag="osb", name="osb")
                    nc.scalar.activation(osb[:], fcps[:, :gpc], AF.Identity,
                                         bias=pk["fcbp"][:, m:m + 1])
                    nc.sync.dma_start(out_d.ap()[m * 128:(m + 1) * 128, :], osb[:])

    nc.compile()
    return nc


def host_prep(inputs, gpc=GPC, n_cores=NCORES):
    """Build per-core in_maps from the full problem inputs."""
    g = lambda k: np.asarray(inputs[k], np.float32)
    A = _host_adjacency(inputs["nodevec1"], inputs["nodevec2"])
    R = SUB_G * N
    abd = np.zeros((128, R), np.float32)
    for b in range(SUB_G):
        abd[b * N:(b + 1) * N, b * N:(b + 1) * N] = A.T   # rhs[j, i] = A[i, j]
    abd = abd.astype(np.float16)

    h = lambda k: np.asarray(inputs[k], np.float32).astype(np.float16)
    shared = {
        "w1": h("W1"), "rs1w": h("rs1_w"), "w2": h("W2"), "w3": h("W3"),
        "fcw": h("fc_w"),
        "sew1": (g("se_w1") / np.float32(N)).astype(np.float16),
        "sew2": h("se_w2"), "abd": abd,
        "b1p": _pack_ch(g("b1"), 8), "bn1gp": _pack_ch(g("bn1_g"), 8),
        "bn1bp": _pack_ch(g("bn1_b"), 8), "rs1bp": _pack_ch(g("rs1_b"), 8),
        "b2p": _pack_ch(g("b2"), 8), "bn2gp": _pack_ch(g("bn2_g"), 8),
        "bn2bp": _pack_ch(g("bn2_b"), 8),
        "b3p": _pack_ch(g("b3"), 4), "bn3gp": _pack_ch(g("bn3_g"), 4),
        "bn3bp": _pack_ch(g("bn3_b"), 4), "fcbp": _pack_ch(g("fc_b"), 8),
    }
    shared = {k: np.ascontiguousarray(v) for k, v in shared.items()}
    x = g("x")
    rows = gpc * N
    in_maps = []
    for i in range(n_cores):
        m = dict(shared)
        m["xT"] = np.ascontiguousarray(x[i * rows:(i + 1) * rows, :].T.astype(np.float16))
        in_maps.append(m)
    return in_maps


_cache = {}


def run(inputs, trace=False, trace_cores=None):
    from concourse.bass_utils import run_bass_kernel_spmd
    key = (GPC, NCORES)
    if key not in _cache:
        _cache[key] = build_nc(GPC, NCORES)
    nc = _cache[key]
    in_maps = host_prep(inputs, GPC, NCORES)
    res = run_bass_kernel_spmd(nc, in_maps, core_ids=list(range(NCORES)),
                               trace=trace, trace_cores=trace_cores)
    shards = [np.asarray(res.results[i]["out"]) for i in range(NCORES)]
    out = np.concatenate([s.T for s in shards], axis=0).astype(np.float32)
    return out, res


def kernel(**inputs) -> np.ndarray:
    out, _ = run(inputs, trace=False)
    return out


# revision 8
# speedup vs baseline: 1.2934x; 1.1274x over previous
"""AdaptiveGCN Trainium2 kernel — 8-core data-parallel over the graph/batch dim.

Layout strategy (per core, channel-major activations):
  - activations live as [C, rows] (channel on partitions), weights as [cin, cout].
  - channel matmul uses the row-major-out form: out[rows, cout] = lhsT(act).T @ rhs(W)
  - message passing uses the channel-major-out form:
      agg[c, i] = lhsT(hm[rows_j, c]).T @ rhs(A_blockdiag[rows_j, rows_i])
    so the layer output lands back in channel-major with zero transposes.
  - channel matmuls run as float32r (full PE rate at free-dim >= 256),
    message-passing matmuls run bf16 (tiny free dim).
  - BatchNorm is training-mode over ALL rows: local Welford stats via
    bn_stats/bn_aggr, merged across the 8 cores with one small AllReduce
    per layer. Activations round-trip HBM (f32) across the BN barriers.
"""

import numpy as np
import ml_dtypes

# problem dims (hardcoded per task spec)
B = 4096
N = 21
CIN = 512
H1 = 1024
H2 = 1024
COUT = 512
FCO = 1024
SEH = 128
RANK = 10
EPS = 1e-5
THRESH = 0.1
NCORES = 8
GPC = B // NCORES            # graphs per core
ST_G = 36                    # graphs per supertile (756 rows)
SUB_G = 6                    # graphs per matmul row-tile (126 rows)


def _host_adjacency(nodevec1, nodevec2):
    """Replicate the reference adjacency computation in f32 numpy."""
    nv1 = np.asarray(nodevec1, np.float32)
    nv2 = np.asarray(nodevec2, np.float32)
    logits = (nv1 @ nv2).astype(np.float32)
    adp = (1.0 / (1.0 + np.exp(-logits, dtype=np.float32))).astype(np.float32)
    at = (adp + np.eye(N, dtype=np.float32)).astype(np.float32)
    deg = at.sum(1, dtype=np.float32)
    dinv = np.where(deg > 0, deg.astype(np.float32) ** -0.5, 0.0).astype(np.float32)
    A = (dinv[:, None] * at * dinv[None, :]).astype(np.float32)
    return np.where(A > THRESH, A, 0.0).astype(np.float32)


def _pack_ch(vec, nch):
    """[nch*128] channel vector -> [128, nch] (partition = channel % 128)."""
    v = np.asarray(vec, np.float32).reshape(nch, 128)
    return np.ascontiguousarray(v.T)


def _supertiles(gpc):
    out, g = [], 0
    while g < gpc:
        gc = min(ST_G, gpc - g)
        out.append((g, gc))
        g += gc
    return out


def _subtiles(gc):
    out, r, rows = [], 0, gc * N
    while r < rows:
        rr = min(SUB_G * N, rows - r)
        rpad = 128 if r + 128 <= rows else rr   # K=128-padded reads for FWL
        out.append((r, rr, rpad))
        out[-1] = (r, rr, rpad)
        r += rr
    return out




def _halves(gc):
    """Group subtiles into <=378-row spans (psum bank / bn_stats limit)."""
    groups, cur, rows = [], [], 0
    for s in _subtiles(gc):
        if rows + s[1] > 378:
            groups.append(cur)
            cur, rows = [], 0
        cur.append(s)
        rows += s[1]
    if cur:
        groups.append(cur)
    out = []
    for g in groups:
        h0 = g[0][0]
        hw = sum(s[1] for s in g)
        out.append((h0, hw, g))
    return out


def build_nc(gpc=GPC, n_cores=NCORES):
    import concourse.bass as bass
    import concourse.bacc as bacc
    import concourse.mybir as mybir
    import concourse.tile as tile

    f32 = mybir.dt.float32
    f16 = mybir.dt.float16
    AF = mybir.ActivationFunctionType
    OP = mybir.AluOpType
    AX = mybir.AxisListType.X

    ROWS = gpc * N
    STR = ST_G * N               # supertile rows (756)
    sts = _supertiles(gpc)
    n_st = len(sts)
    n_slots = sum(len(_halves(gc)) for _, gc in sts)
    rg = [list(range(n_cores))]

    nc = bacc.Bacc("TRN2", target_bir_lowering=False, debug=False,
                   num_devices=n_cores)

    # ---- external I/O -------------------------------------------------
    xT = nc.dram_tensor("xT", [CIN, ROWS], f16, kind="ExternalInput")
    w1_d = nc.dram_tensor("w1", [CIN, H1], f16, kind="ExternalInput")
    rs1_d = nc.dram_tensor("rs1w", [CIN, H1], f16, kind="ExternalInput")
    w2_d = nc.dram_tensor("w2", [H1, H2], f16, kind="ExternalInput")
    w3_d = nc.dram_tensor("w3", [H2, COUT], f16, kind="ExternalInput")
    fcw_d = nc.dram_tensor("fcw", [COUT, FCO], f16, kind="ExternalInput")
    sew1_d = nc.dram_tensor("sew1", [CIN, SEH], f16, kind="ExternalInput")  # pre-scaled 1/21
    sew2_d = nc.dram_tensor("sew2", [SEH, CIN], f16, kind="ExternalInput")
    abd_d = nc.dram_tensor("abd", [128, SUB_G * N], f16, kind="ExternalInput")
    pk_d = {}
    for nm, nch in [("b1p", 8), ("bn1gp", 8), ("bn1bp", 8), ("rs1bp", 8),
                    ("b2p", 8), ("bn2gp", 8), ("bn2bp", 8),
                    ("b3p", 4), ("bn3gp", 4), ("bn3bp", 4), ("fcbp", 8)]:
        pk_d[nm] = nc.dram_tensor(nm, [128, nch], f32, kind="ExternalInput")
    out_d = nc.dram_tensor("out", [FCO, gpc], f32, kind="ExternalOutput")

    with tile.TileContext(nc) as tc:
        with tc.tile_pool(name="persist", bufs=1) as pp, \
             tc.tile_pool(name="scratch", bufs=2) as scp, \
             tc.tile_pool(name="dram", bufs=1, space="DRAM") as dmp, \
             tc.tile_pool(name="ps_hm", bufs=2, space="PSUM") as ps_hm, \
             tc.tile_pool(name="ps_small", bufs=4, space="PSUM") as ps_s:

            # DRAM spill buffers (f16, 1512B per-partition lines at ST_G=36)
            z1_t = dmp.tile([H1, ROWS], f16, tag="z1")
            xg_t = dmp.tile([CIN, ROWS], f16, tag="xgsp")
            u_t = dmp.tile([H1, ROWS], f16, tag="usp")
            z2_t = dmp.tile([H2, ROWS], f16, tag="z2")
            stb_in = {l: dmp.tile([128, 2 * nch], f32, tag=f"stbi{l}", name=f"stbi{l}")
                      for l, nch in [(1, 8), (2, 8), (3, 4)]}
            stb_out = {l: dmp.tile([128, 2 * nch], f32, tag=f"stbo{l}", name=f"stbo{l}")
                       for l, nch in [(1, 8), (2, 8), (3, 4)]}

            # packed per-channel params
            pk = {}
            for nm, d in pk_d.items():
                t = pp.tile(list(d.shape), f32, tag=nm, name=nm)
                nc.sync.dma_start(t[:], d.ap()[:])
                pk[nm] = t
            abd_sb = pp.tile([128, SUB_G * N], f16, tag="abd")
            nc.sync.dma_start(abd_sb[:], abd_d.ap()[:])

            bnsl = {1: [pp.tile([128, n_slots * 6], f32, tag=f"bnsl1_{c}", name=f"bnsl1_{c}") for c in range(8)],
                    2: [pp.tile([128, n_slots * 6], f32, tag=f"bnsl2_{c}", name=f"bnsl2_{c}") for c in range(8)],
                    3: [pp.tile([128, n_slots * 6], f32, tag=f"bnsl3_{c}", name=f"bnsl3_{c}") for c in range(4)]}
            pool_sb = [pp.tile([128, gpc], f32, tag=f"pool_{c}", name=f"pool_{c}") for c in range(4)]
            sv = {1: pp.tile([128, 8], f32, tag="s1v", name="s1v"),
                  2: pp.tile([128, 8], f32, tag="s2v", name="s2v"),
                  3: pp.tile([128, 4], f32, tag="s3v", name="s3v")}
            tv = {1: pp.tile([128, 8], f32, tag="t1v", name="t1v"),
                  2: pp.tile([128, 8], f32, tag="t2v", name="t2v"),
                  3: pp.tile([128, 4], f32, tag="t3v", name="t3v")}
            s3p = pp.tile([128, 4], f32, tag="s3p")   # s3 / 21 for pooled affine

            def finalize_stats(l, nch, gp, bp, extra_bias=None):
                """bn slots -> AllReduce -> affine params sv[l], tv[l]."""
                loc = scp.tile([128, 2 * nch], f32, tag=f"loc{l}", name=f"loc{l}")
                for c in range(nch):
                    ag = scp.tile([128, 2], f32, tag="bnag", name="bnag")
                    nc.vector.bn_aggr(ag[:], bnsl[l][c][:])
                    nc.vector.tensor_copy(loc[:, c:c + 1], ag[:, 0:1])
                    # Ez2 = mean*mean + var
                    nc.vector.scalar_tensor_tensor(
                        loc[:, nch + c:nch + c + 1], ag[:, 0:1], ag[:, 0:1],
                        ag[:, 1:2], op0=OP.mult, op1=OP.add)
                nc.sync.dma_start(stb_in[l][:], loc[:])
                nc.gpsimd.collective_compute(
                    "AllReduce", OP.add, replica_groups=rg,
                    ins=[stb_in[l][:].opt()], outs=[stb_out[l][:].opt()])
                red = scp.tile([128, 2 * nch], f32, tag=f"red{l}", name=f"red{l}")
                nc.sync.dma_start(red[:], stb_out[l][:])
                mg = scp.tile([128, nch], f32, tag=f"mg{l}", name=f"mg{l}")
                e2 = scp.tile([128, nch], f32, tag=f"e2{l}", name=f"e2{l}")
                nc.vector.tensor_scalar_mul(mg[:], red[:, :nch], 1.0 / n_cores)
                nc.vector.tensor_scalar_mul(e2[:], red[:, nch:], 1.0 / n_cores)
                var = scp.tile([128, nch], f32, tag=f"var{l}", name=f"var{l}")
                nc.vector.tensor_tensor(var[:], mg[:], mg[:], op=OP.mult)
                nc.vector.tensor_tensor(var[:], e2[:], var[:], op=OP.subtract)
                nc.vector.tensor_scalar_add(var[:], var[:], EPS)
                sq = scp.tile([128, nch], f32, tag=f"sq{l}", name=f"sq{l}")
                nc.scalar.activation(sq[:], var[:], AF.Sqrt)
                y0 = scp.tile([128, nch], f32, tag=f"y0{l}", name=f"y0{l}")
                nc.vector.reciprocal(y0[:], sq[:])
                # one Newton step: y1 = y0 * (1.5 - 0.5 * var * y0^2)
                yy = scp.tile([128, nch], f32, tag=f"yy{l}", name=f"yy{l}")
                nc.vector.tensor_tensor(yy[:], y0[:], y0[:], op=OP.mult)
                nc.vector.tensor_tensor(yy[:], var[:], yy[:], op=OP.mult)
                nc.vector.tensor_scalar(yy[:], yy[:], -0.5, 1.5,
                                        op0=OP.mult, op1=OP.add)
                nc.vector.tensor_tensor(y0[:], y0[:], yy[:], op=OP.mult)
                nc.vector.tensor_tensor(sv[l][:, :nch], gp[:], y0[:], op=OP.mult)
                ms = scp.tile([128, nch], f32, tag=f"ms{l}", name=f"ms{l}")
                nc.vector.tensor_tensor(ms[:], mg[:], sv[l][:, :nch], op=OP.mult)
                nc.vector.tensor_tensor(tv[l][:, :nch], bp[:], ms[:], op=OP.subtract)
                if extra_bias is not None:
                    nc.vector.tensor_tensor(tv[l][:, :nch], tv[l][:, :nch],
                                            extra_bias[:], op=OP.add)

            def cast_hm(si, dst, src):
                # alternate engines so neither DVE nor ACT owns all casts
                if si % 2 == 0:
                    nc.vector.tensor_copy(dst, src)
                else:
                    nc.scalar.copy(dst, src)

            def mp_layer(st_slot0, gc, hm_list, nch_out, zsb, bias_pk, lsl):
                """c-outer message passing + bias + bn_stats for one supertile."""
                for c in range(nch_out):
                    for hi, (h0, hw, subs) in enumerate(_halves(gc)):
                        aggps = ps_s.tile([128, 512], f32, tag="small", name="aggps")
                        for (r0, rr, rp) in subs:
                            hm_sb = hm_list[[s[0] for s in _subtiles(gc)].index(r0)]
                            nc.tensor.matmul(aggps[:, r0 - h0:r0 - h0 + rr],
                                             lhsT=hm_sb[:rp, c * 128:(c + 1) * 128],
                                             rhs=abd_sb[:rp, :rr],
                                             start=True, stop=True)
                        nc.scalar.activation(zsb[c][:, h0:h0 + hw], aggps[:, :hw],
                                             AF.Identity, bias=bias_pk[:, c:c + 1])
                        slot = st_slot0 + hi
                        nc.vector.bn_stats(lsl[c][:, slot * 6:(slot + 1) * 6],
                                           zsb[c][:, h0:h0 + hw])

            # ================= Phase A: SE gate + layer 1 + xg spill ====
            with tc.tile_pool(name="wA", bufs=1) as wp, \
                 tc.tile_pool(name="actA", bufs=2) as ap_:
                w1_sb = []
                sew1_sb = []
                for k in range(4):
                    t = wp.tile([128, H1], f16, tag=f"w1_{k}", name=f"w1sb_{k}")
                    nc.sync.dma_start(t[:], w1_d.ap()[k * 128:(k + 1) * 128, :])
                    w1_sb.append(t)
                    t = wp.tile([128, SEH], f16, tag=f"sew1_{k}", name=f"sew1sb_{k}")
                    nc.sync.dma_start(t[:], sew1_d.ap()[k * 128:(k + 1) * 128, :])
                    sew1_sb.append(t)
                sew2_sb = wp.tile([128, CIN], f16, tag="sew2")
                nc.sync.dma_start(sew2_sb[:], sew2_d.ap()[:])

                st_slot = 0
                for sti, (g0, gc) in enumerate(sts):
                    rows = gc * N
                    c0 = g0 * N
                    x_sb = []
                    for k in range(4):
                        t = ap_.tile([128, STR], f16, tag=f"x_{k}", name=f"x_{k}")
                        nc.sync.dma_start(t[:, :rows],
                                          xT.ap()[k * 128:(k + 1) * 128, c0:c0 + rows])
                        x_sb.append(t)
                    # SE: node-sum (1/21 folded into sew1) -> 2 tiny mms
                    xm = []
                    for k in range(4):
                        t = ap_.tile([128, ST_G], f16, tag=f"xm_{k}", name=f"xm_{k}")
                        with nc.allow_low_precision(reason="fp16 rounding for SE matmul"):
                            nc.vector.tensor_reduce(
                                t[:, :gc],
                                x_sb[k][:, :rows].rearrange("p (g n) -> p g n", n=N),
                                axis=AX, op=OP.add)
                        xm.append(t)
                    y1ps = ps_s.tile([128, 512], f32, tag="small", name="y1ps")
                    for k in range(4):
                        nc.tensor.matmul(y1ps[:, :gc], lhsT=sew1_sb[k][:],
                                         rhs=xm[k][:, :gc],
                                         start=(k == 0), stop=(k == 3))
                    y1_sb = ap_.tile([128, ST_G], f16, tag="y1", name="y1")
                    nc.scalar.activation(y1_sb[:, :gc], y1ps[:, :gc], AF.Relu)
                    y_sb = []
                    for m in range(4):
                        y2ps = ps_s.tile([128, 512], f32, tag="small", name="y2ps")
                        nc.tensor.matmul(y2ps[:, :gc],
                                         lhsT=sew2_sb[:, m * 128:(m + 1) * 128],
                                         rhs=y1_sb[:, :gc], start=True, stop=True)
                        t = ap_.tile([128, ST_G], f16, tag=f"y_{m}", name=f"y_{m}")
                        nc.scalar.activation(t[:, :gc], y2ps[:, :gc], AF.Sigmoid)
                        y_sb.append(t)
                    # gate + xg spill
                    xg = []
                    for k in range(4):
                        t = ap_.tile([128, STR], f16, tag=f"xg_{k}", name=f"xg_{k}")
                        nc.vector.tensor_tensor(
                            t[:, :rows].rearrange("p (g n) -> p g n", n=N),
                            x_sb[k][:, :rows].rearrange("p (g n) -> p g n", n=N),
                            y_sb[k][:, :gc].broadcast_to([128, gc, N]),
                            op=OP.mult)
                        nc.sync.dma_start(xg_t[k * 128:(k + 1) * 128, c0:c0 + rows],
                                          t[:, :rows])
                        xg.append(t)
                    # layer-1 channel matmuls
                    hm_list = []
                    for si, (r0, rr, rp) in enumerate(_subtiles(gc)):
                        hmps = ps_hm.tile([128, H1], f32, tag="hm", name="hmps")
                        for k in range(4):
                            for n2 in range(2):
                                nc.tensor.matmul(
                                    hmps[:rp, n2 * 512:(n2 + 1) * 512],
                                    lhsT=xg[k][:, r0:r0 + rp],
                                    rhs=w1_sb[k][:, n2 * 512:(n2 + 1) * 512],
                                    start=(k == 0), stop=(k == 3))
                        hm_sb = ap_.tile([128, H1], f16, tag="hmsb", name="hm_sb",
                                         bufs=7)
                        cast_hm(si, hm_sb[:rp, :], hmps[:rp, :])
                        hm_list.append(hm_sb)
                    z1sb = [ap_.tile([128, STR], f16, tag=f"z1sb_{c}", name=f"z1sb_{c}")
                            for c in range(8)]
                    mp_layer(st_slot, gc, hm_list, 8, z1sb, pk["b1p"], bnsl[1])
                    for c in range(8):
                        nc.sync.dma_start(z1_t[c * 128:(c + 1) * 128, c0:c0 + rows],
                                          z1sb[c][:, :rows])
                    st_slot += len(_halves(gc))

                finalize_stats(1, 8, pk["bn1gp"], pk["bn1bp"], pk["rs1bp"])

            # ======== Phase B: junction 1 (+r1 mm from psum) + layer 2 ==
            with tc.tile_pool(name="wB", bufs=1) as wp, \
                 tc.tile_pool(name="actB", bufs=2) as ap_:
                w2_sb = []
                for k in range(8):
                    t = wp.tile([128, H2], f16, tag=f"w2_{k}", name=f"w2sb_{k}")
                    nc.sync.dma_start(t[:], w2_d.ap()[k * 128:(k + 1) * 128, :])
                    w2_sb.append(t)
                rs1_sb = []
                for k in range(4):
                    t = wp.tile([128, H1], f16, tag=f"rs1_{k}", name=f"rs1sb_{k}")
                    nc.sync.dma_start(t[:], rs1_d.ap()[k * 128:(k + 1) * 128, :])
                    rs1_sb.append(t)
                st_slot = 0
                for sti, (g0, gc) in enumerate(sts):
                    rows = gc * N
                    c0 = g0 * N
                    xgr = []
                    for k in range(4):
                        t = ap_.tile([128, STR], f16, tag=f"xgr_{k}", name=f"xgr_{k}")
                        nc.sync.dma_start(t[:, :rows],
                                          xg_t[k * 128:(k + 1) * 128, c0:c0 + rows])
                        xgr.append(t)
                    h1 = []
                    for c in range(8):
                        z1r = ap_.tile([128, STR], f16, tag=f"z1r_{c}", name=f"z1r_{c}")
                        nc.sync.dma_start(z1r[:, :rows],
                                          z1_t[c * 128:(c + 1) * 128, c0:c0 + rows])
                        ut = ap_.tile([128, STR], f16, tag=f"ut_{c}", name=f"ut_{c}")
                        for (h0, hw, subs) in _halves(gc):
                            r1ps = ps_s.tile([128, 512], f32, tag="small", name="r1ps")
                            for k in range(4):
                                nc.tensor.matmul(r1ps[:, :hw],
                                                 lhsT=rs1_sb[k][:, c * 128:(c + 1) * 128],
                                                 rhs=xgr[k][:, h0:h0 + hw],
                                                 start=(k == 0), stop=(k == 3))
                            nc.vector.scalar_tensor_tensor(
                                ut[:, h0:h0 + hw], z1r[:, h0:h0 + hw],
                                sv[1][:, c:c + 1], r1ps[:, :hw],
                                op0=OP.mult, op1=OP.add)
                        nc.sync.dma_start(u_t[c * 128:(c + 1) * 128, c0:c0 + rows],
                                          ut[:, :rows])
                        t = ap_.tile([128, STR], f16, tag=f"h1_{c}", name=f"h1_{c}")
                        nc.scalar.activation(t[:, :rows], ut[:, :rows], AF.Relu,
                                             bias=tv[1][:, c:c + 1])
                        h1.append(t)
                    hm_list = []
                    for si, (r0, rr, rp) in enumerate(_subtiles(gc)):
                        hmps = ps_hm.tile([128, H2], f32, tag="hm", name="hmps2")
                        for k in range(8):
                            for n2 in range(2):
                                nc.tensor.matmul(
                                    hmps[:rp, n2 * 512:(n2 + 1) * 512],
                                    lhsT=h1[k][:, r0:r0 + rp],
                                    rhs=w2_sb[k][:, n2 * 512:(n2 + 1) * 512],
                                    start=(k == 0), stop=(k == 7))
                        hm_sb = ap_.tile([128, H2], f16, tag="hmsb", name="hm_sb2",
                                         bufs=7)
                        cast_hm(si, hm_sb[:rp, :], hmps[:rp, :])
                        hm_list.append(hm_sb)
                    z2sb = [ap_.tile([128, STR], f16, tag=f"z2sb_{c}", name=f"z2sb_{c}")
                            for c in range(8)]
                    mp_layer(st_slot, gc, hm_list, 8, z2sb, pk["b2p"], bnsl[2])
                    for c in range(8):
                        nc.sync.dma_start(z2_t[c * 128:(c + 1) * 128, c0:c0 + rows],
                                          z2sb[c][:, :rows])
                    st_slot += len(_halves(gc))
                finalize_stats(2, 8, pk["bn2gp"], pk["bn2bp"])

            # ================= Phase C: junction 2 + layer 3 + pool =====
            with tc.tile_pool(name="wC", bufs=1) as wp, \
                 tc.tile_pool(name="actC", bufs=2) as ap_:
                w3_sb = []
                for k in range(8):
                    t = wp.tile([128, COUT], f16, tag=f"w3_{k}", name=f"w3sb_{k}")
                    nc.sync.dma_start(t[:], w3_d.ap()[k * 128:(k + 1) * 128, :])
                    w3_sb.append(t)
                st_slot = 0
                for sti, (g0, gc) in enumerate(sts):
                    rows = gc * N
                    c0 = g0 * N
                    h2 = []
                    for c in range(8):
                        z2r = ap_.tile([128, STR], f16, tag=f"z2r_{c}", name=f"z2r_{c}")
                        nc.sync.dma_start(z2r[:, :rows],
                                          z2_t[c * 128:(c + 1) * 128, c0:c0 + rows])
                        ur = ap_.tile([128, STR], f16, tag=f"ur_{c}", name=f"ur_{c}")
                        nc.sync.dma_start(ur[:, :rows],
                                          u_t[c * 128:(c + 1) * 128, c0:c0 + rows])
                        h1c = ap_.tile([128, STR], f16, tag=f"h1c_{c}", name=f"h1c_{c}")
                        nc.scalar.activation(h1c[:, :rows], ur[:, :rows], AF.Relu,
                                             bias=tv[1][:, c:c + 1])
                        bt = ap_.tile([128, STR], f16, tag=f"bt_{c}", name=f"bt_{c}")
                        nc.vector.scalar_tensor_tensor(
                            bt[:, :rows], z2r[:, :rows], sv[2][:, c:c + 1],
                            h1c[:, :rows], op0=OP.mult, op1=OP.add)
                        t = ap_.tile([128, STR], f16, tag=f"h2_{c}", name=f"h2_{c}")
                        nc.scalar.activation(t[:, :rows], bt[:, :rows], AF.Relu,
                                             bias=tv[2][:, c:c + 1])
                        h2.append(t)
                    hm_list = []
                    for si, (r0, rr, rp) in enumerate(_subtiles(gc)):
                        hmps = ps_hm.tile([128, H2], f32, tag="hm", name="hmps3")
                        for k in range(8):
                            nc.tensor.matmul(hmps[:rp, :COUT],
                                             lhsT=h2[k][:, r0:r0 + rp],
                                             rhs=w3_sb[k][:],
                                             start=(k == 0), stop=(k == 7))
                        hm_sb = ap_.tile([128, H2], f16, tag="hmsb", name="hm_sb3",
                                         bufs=7)
                        cast_hm(si, hm_sb[:rp, :COUT], hmps[:rp, :COUT])
                        hm_list.append(hm_sb)
                    z3sb = [ap_.tile([128, STR], f16, tag=f"z3sb_{c}", name=f"z3sb_{c}")
                            for c in range(4)]
                    mp_layer(st_slot, gc, hm_list, 4, z3sb, pk["b3p"], bnsl[3])
                    for c in range(4):
                        nc.vector.tensor_reduce(
                            pool_sb[c][:, g0:g0 + gc],
                            z3sb[c][:, :rows].rearrange("p (g n) -> p g n", n=N),
                            axis=AX, op=OP.add)
                    st_slot += len(_halves(gc))
                finalize_stats(3, 4, pk["bn3gp"], pk["bn3bp"])
                nc.vector.tensor_scalar_mul(s3p[:], sv[3][:], 1.0 / N)

            # ================= Phase D: pooled affine + FC ==============
            with tc.tile_pool(name="wD", bufs=1) as wp, \
                 tc.tile_pool(name="actD", bufs=2) as ap_:
                fcw_sb = []
                for k in range(4):
                    t = wp.tile([128, FCO], f16, tag=f"fcw_{k}", name=f"fcwsb_{k}")
                    nc.sync.dma_start(t[:], fcw_d.ap()[k * 128:(k + 1) * 128, :])
                    fcw_sb.append(t)
                pbn = []
                for c in range(4):
                    t = ap_.tile([128, gpc], f16, tag=f"pbn_{c}", name=f"pbn_{c}")
                    nc.vector.tensor_scalar(t[:], pool_sb[c][:],
                                            s3p[:, c:c + 1], tv[3][:, c:c + 1],
                                            op0=OP.mult, op1=OP.add)
                    pbn.append(t)
                for m in range(8):
                    fcps = ps_s.tile([128, 512], f32, tag="small", name="fcps")
                    for k in range(4):
                        nc.tensor.matmul(fcps[:, :gpc],
                                         lhsT=fcw_sb[k][:, m * 128:(m + 1) * 128],
                                         rhs=pbn[k][:],
                                         start=(k == 0), stop=(k == 3))
                    osb = ap_.tile([128, gpc], f32, tag="osb", name="osb")
                    nc.scalar.activation(osb[:], fcps[:, :gpc], AF.Identity,
                                         bias=pk["fcbp"][:, m:m + 1])
                    nc.sync.dma_start(out_d.ap()[m * 128:(m + 1) * 128, :], osb[:])

    nc.compile()
    return nc


def host_prep(inputs, gpc=GPC, n_cores=NCORES):
    """Build per-core in_maps from the full problem inputs."""
    g = lambda k: np.asarray(inputs[k], np.float32)
    A = _host_adjacency(inputs["nodevec1"], inputs["nodevec2"])
    R = SUB_G * N
    abd = np.zeros((128, R), np.float32)
    for b in range(SUB_G):
        abd[b * N:(b + 1) * N, b * N:(b + 1) * N] = A.T   # rhs[j, i] = A[i, j]
    abd = abd.astype(np.float16)

    h = lambda k: np.asarray(inputs[k], np.float32).astype(np.float16)
    shared = {
        "w1": h("W1"), "rs1w": h("rs1_w"), "w2": h("W2"), "w3": h("W3"),
        "fcw": h("fc_w"),
        "sew1": (g("se_w1") / np.float32(N)).astype(np.float16),
        "sew2": h("se_w2"), "abd": abd,
        "b1p": _pack_ch(g("b1"), 8), "bn1gp": _pack_ch(g("bn1_g"), 8),
        "bn1bp": _pack_ch(g("bn1_b"), 8), "rs1bp": _pack_ch(g("rs1_b"), 8),
        "b2p": _pack_ch(g("b2"), 8), "bn2gp": _pack_ch(g("bn2_g"), 8),
        "bn2bp": _pack_ch(g("bn2_b"), 8),
        "b3p": _pack_ch(g("b3"), 4), "bn3gp": _pack_ch(g("bn3_g"), 4),
        "bn3bp": _pack_ch(g("bn3_b"), 4), "fcbp": _pack_ch(g("fc_b"), 8),
    }
    shared = {k: np.ascontiguousarray(v) for k, v in shared.items()}
    x = g("x")
    rows = gpc * N
    in_maps = []
    for i in range(n_cores):
        m = dict(shared)
        m["xT"] = np.ascontiguousarray(x[i * rows:(i + 1) * rows, :].T.astype(np.float16))
        in_maps.append(m)
    return in_maps


_cache = {}


def run(inputs, trace=False, trace_cores=None):
    from concourse.bass_utils import run_bass_kernel_spmd
    key = (GPC, NCORES)
    if key not in _cache:
        _cache[key] = build_nc(GPC, NCORES)
    nc = _cache[key]
    in_maps = host_prep(inputs, GPC, NCORES)
    res = run_bass_kernel_spmd(nc, in_maps, core_ids=list(range(NCORES)),
                               trace=trace, trace_cores=trace_cores)
    shards = [np.asarray(res.results[i]["out"]) for i in range(NCORES)]
    out = np.concatenate([s.T for s in shards], axis=0).astype(np.float32)
    return out, res


def kernel(**inputs) -> np.ndarray:
    out, _ = run(inputs, trace=False)
    return out


# revision 10
# speedup vs baseline: 1.5531x; 1.2008x over previous
"""AdaptiveGCN Trainium2 kernel — 8-core data-parallel over the graph/batch dim.

Layout strategy (per core, channel-major activations):
  - activations live as [C, rows] (channel on partitions), weights as [cin, cout].
  - channel matmul uses the row-major-out form: out[rows, cout] = lhsT(act).T @ rhs(W)
  - message passing uses the channel-major-out form:
      agg[c, i] = lhsT(hm[rows_j, c]).T @ rhs(A_blockdiag[rows_j, rows_i])
    so the layer output lands back in channel-major with zero transposes.
  - channel matmuls run as float32r (full PE rate at free-dim >= 256),
    message-passing matmuls run bf16 (tiny free dim).
  - BatchNorm is training-mode over ALL rows: local Welford stats via
    bn_stats/bn_aggr, merged across the 8 cores with one small AllReduce
    per layer. Activations round-trip HBM (f32) across the BN barriers.
"""

import numpy as np
import ml_dtypes

# problem dims (hardcoded per task spec)
B = 4096
N = 21
CIN = 512
H1 = 1024
H2 = 1024
COUT = 512
FCO = 1024
SEH = 128
RANK = 10
EPS = 1e-5
THRESH = 0.1
NCORES = 8
GPC = B // NCORES            # graphs per core
ST_G = 36                    # graphs per supertile (756 rows)
SUB_G = 6                    # graphs per matmul row-tile (126 rows)


def _host_adjacency(nodevec1, nodevec2):
    """Replicate the reference adjacency computation in f32 numpy."""
    nv1 = np.asarray(nodevec1, np.float32)
    nv2 = np.asarray(nodevec2, np.float32)
    logits = (nv1 @ nv2).astype(np.float32)
    adp = (1.0 / (1.0 + np.exp(-logits, dtype=np.float32))).astype(np.float32)
    at = (adp + np.eye(N, dtype=np.float32)).astype(np.float32)
    deg = at.sum(1, dtype=np.float32)
    dinv = np.where(deg > 0, deg.astype(np.float32) ** -0.5, 0.0).astype(np.float32)
    A = (dinv[:, None] * at * dinv[None, :]).astype(np.float32)
    return np.where(A > THRESH, A, 0.0).astype(np.float32)


def _pack_ch(vec, nch):
    """[nch*128] channel vector -> [128, nch] (partition = channel % 128)."""
    v = np.asarray(vec, np.float32).reshape(nch, 128)
    return np.ascontiguousarray(v.T)


def _supertiles(gpc):
    out, g = [], 0
    while g < gpc:
        gc = min(ST_G, gpc - g)
        out.append((g, gc))
        g += gc
    return out


def _subtiles(gc):
    out, r, rows = [], 0, gc * N
    while r < rows:
        rr = min(SUB_G * N, rows - r)
        rpad = 128 if r + 128 <= rows else rr   # K=128-padded reads for FWL
        out.append((r, rr, rpad))
        out[-1] = (r, rr, rpad)
        r += rr
    return out




def _halves(gc):
    """Group subtiles into <=378-row spans (psum bank / bn_stats limit)."""
    groups, cur, rows = [], [], 0
    for s in _subtiles(gc):
        if rows + s[1] > 378:
            groups.append(cur)
            cur, rows = [], 0
        cur.append(s)
        rows += s[1]
    if cur:
        groups.append(cur)
    out = []
    for g in groups:
        h0 = g[0][0]
        hw = sum(s[1] for s in g)
        out.append((h0, hw, g))
    return out



def _dedup_ldweights(nc, mybir):
    """Drop consecutive identical LDWEIGHTS in the PE stream (walrus ldw-opt
    is disabled, so every matmul otherwise reloads its stationary operand)."""
    def sig_of(ins):
        a = ins.ins[0]
        ap = getattr(a, "bass_ap", None)
        if ap is None:
            return None
        try:
            return (ap.tensor.name, ap.offset, tuple(map(tuple, ap.ap)), str(ap.dtype))
        except Exception:
            return None

    import os
    if os.environ.get("GCN_NO_DEDUP"):
        return 0
    dropped = 0
    for bb in nc.main_func.blocks:
        keep = []
        last = None
        pending = None   # dup LDW awaiting its matmul's wait check
        for ins in bb.instructions:
            if isinstance(ins, mybir.InstLdweights):
                if pending is not None:
                    keep.append(pending)
                    pending = None
                sig = sig_of(ins)
                clean = not (ins.sync_info and
                             (ins.sync_info.on_wait or ins.sync_info.on_update))
                if sig is not None and sig == last and clean:
                    pending = ins   # drop only if the next matmul is wait-free
                    continue
                last = sig
                keep.append(ins)
                continue
            if isinstance(ins, mybir.InstMatmult):
                if pending is not None:
                    mm_clean = not (ins.sync_info and ins.sync_info.on_wait)
                    if mm_clean:
                        dropped += 1
                    else:
                        keep.append(pending)
                    pending = None
                keep.append(ins)
                continue
            if pending is not None:
                keep.append(pending)
                pending = None
            if getattr(ins, "engine", None) == mybir.EngineType.PE:
                last = None
            if last is not None:
                for a in ins.outs:
                    ap = getattr(a, "bass_ap", None)
                    if ap is not None and ap.tensor.name == last[0]:
                        last = None
                        break
            keep.append(ins)
        if pending is not None:
            keep.append(pending)
        bb.instructions[:] = keep
    return dropped


def build_nc(gpc=GPC, n_cores=NCORES):
    import concourse.bass as bass
    import concourse.bacc as bacc
    import concourse.mybir as mybir
    import concourse.tile as tile

    f32 = mybir.dt.float32
    f16 = mybir.dt.float16
    AF = mybir.ActivationFunctionType
    OP = mybir.AluOpType
    AX = mybir.AxisListType.X

    ROWS = gpc * N
    STR = ST_G * N               # supertile rows (756)
    sts = _supertiles(gpc)
    n_st = len(sts)
    n_slots = sum(len(_halves(gc)) for _, gc in sts)
    rg = [list(range(n_cores))]

    nc = bacc.Bacc("TRN2", target_bir_lowering=False, debug=False,
                   num_devices=n_cores)

    # ---- external I/O -------------------------------------------------
    xT = nc.dram_tensor("xT", [CIN, ROWS], f16, kind="ExternalInput")
    w1_d = nc.dram_tensor("w1", [CIN, H1], f16, kind="ExternalInput")
    rs1_d = nc.dram_tensor("rs1w", [CIN, H1], f16, kind="ExternalInput")
    w2_d = nc.dram_tensor("w2", [H1, H2], f16, kind="ExternalInput")
    w3_d = nc.dram_tensor("w3", [H2, COUT], f16, kind="ExternalInput")
    fcw_d = nc.dram_tensor("fcw", [COUT, FCO], f16, kind="ExternalInput")
    sew1_d = nc.dram_tensor("sew1", [CIN, SEH], f16, kind="ExternalInput")  # pre-scaled 1/21
    sew2_d = nc.dram_tensor("sew2", [SEH, CIN], f16, kind="ExternalInput")
    abd_d = nc.dram_tensor("abd", [128, SUB_G * N], f16, kind="ExternalInput")
    pk_d = {}
    for nm, nch in [("b1p", 8), ("bn1gp", 8), ("bn1bp", 8), ("rs1bp", 8),
                    ("b2p", 8), ("bn2gp", 8), ("bn2bp", 8),
                    ("b3p", 4), ("bn3gp", 4), ("bn3bp", 4), ("fcbp", 8)]:
        pk_d[nm] = nc.dram_tensor(nm, [128, nch], f32, kind="ExternalInput")
    out_d = nc.dram_tensor("out", [FCO, gpc], f32, kind="ExternalOutput")

    with tile.TileContext(nc) as tc:
        with tc.tile_pool(name="persist", bufs=1) as pp, \
             tc.tile_pool(name="scratch", bufs=2) as scp, \
             tc.tile_pool(name="dram", bufs=1, space="DRAM") as dmp, \
             tc.tile_pool(name="ps_hm", bufs=2, space="PSUM") as ps_hm, \
             tc.tile_pool(name="ps_small", bufs=4, space="PSUM") as ps_s:

            # DRAM spill buffers (f16, 1512B per-partition lines at ST_G=36)
            z1_t = dmp.tile([H1, ROWS], f16, tag="z1")
            xg_t = dmp.tile([CIN, ROWS], f16, tag="xgsp")
            h1_t = dmp.tile([H1, ROWS], f16, tag="h1sp")
            z2_t = dmp.tile([H2, ROWS], f16, tag="z2")
            stb_in = {l: dmp.tile([128, 2 * nch], f32, tag=f"stbi{l}", name=f"stbi{l}")
                      for l, nch in [(1, 8), (2, 8), (3, 4)]}
            stb_out = {l: dmp.tile([128, 2 * nch], f32, tag=f"stbo{l}", name=f"stbo{l}")
                       for l, nch in [(1, 8), (2, 8), (3, 4)]}

            # packed per-channel params
            pk = {}
            for nm, d in pk_d.items():
                t = pp.tile(list(d.shape), f32, tag=nm, name=nm)
                nc.sync.dma_start(t[:], d.ap()[:])
                pk[nm] = t
            abd_sb = pp.tile([128, SUB_G * N], f16, tag="abd")
            nc.sync.dma_start(abd_sb[:], abd_d.ap()[:])

            bnsl = {1: [pp.tile([128, n_slots * 6], f32, tag=f"bnsl1_{c}", name=f"bnsl1_{c}") for c in range(8)],
                    2: [pp.tile([128, n_slots * 6], f32, tag=f"bnsl2_{c}", name=f"bnsl2_{c}") for c in range(8)],
                    3: [pp.tile([128, n_slots * 6], f32, tag=f"bnsl3_{c}", name=f"bnsl3_{c}") for c in range(4)]}
            pool_sb = [pp.tile([128, gpc], f32, tag=f"pool_{c}", name=f"pool_{c}") for c in range(4)]
            sv = {1: pp.tile([128, 8], f32, tag="s1v", name="s1v"),
                  2: pp.tile([128, 8], f32, tag="s2v", name="s2v"),
                  3: pp.tile([128, 4], f32, tag="s3v", name="s3v")}
            tv = {1: pp.tile([128, 8], f32, tag="t1v", name="t1v"),
                  2: pp.tile([128, 8], f32, tag="t2v", name="t2v"),
                  3: pp.tile([128, 4], f32, tag="t3v", name="t3v")}
            s3p = pp.tile([128, 4], f32, tag="s3p")   # s3 / 21 for pooled affine

            def finalize_stats(l, nch, gp, bp, extra_bias=None):
                """bn slots -> AllReduce -> affine params sv[l], tv[l]."""
                loc = scp.tile([128, 2 * nch], f32, tag=f"loc{l}", name=f"loc{l}")
                for c in range(nch):
                    ag = scp.tile([128, 2], f32, tag="bnag", name="bnag")
                    nc.vector.bn_aggr(ag[:], bnsl[l][c][:])
                    nc.vector.tensor_copy(loc[:, c:c + 1], ag[:, 0:1])
                    # Ez2 = mean*mean + var
                    nc.vector.scalar_tensor_tensor(
                        loc[:, nch + c:nch + c + 1], ag[:, 0:1], ag[:, 0:1],
                        ag[:, 1:2], op0=OP.mult, op1=OP.add)
                nc.sync.dma_start(stb_in[l][:], loc[:])
                nc.gpsimd.collective_compute(
                    "AllReduce", OP.add, replica_groups=rg,
                    ins=[stb_in[l][:].opt()], outs=[stb_out[l][:].opt()])
                red = scp.tile([128, 2 * nch], f32, tag=f"red{l}", name=f"red{l}")
                nc.sync.dma_start(red[:], stb_out[l][:])
                mg = scp.tile([128, nch], f32, tag=f"mg{l}", name=f"mg{l}")
                e2 = scp.tile([128, nch], f32, tag=f"e2{l}", name=f"e2{l}")
                nc.vector.tensor_scalar_mul(mg[:], red[:, :nch], 1.0 / n_cores)
                nc.vector.tensor_scalar_mul(e2[:], red[:, nch:], 1.0 / n_cores)
                var = scp.tile([128, nch], f32, tag=f"var{l}", name=f"var{l}")
                nc.vector.tensor_tensor(var[:], mg[:], mg[:], op=OP.mult)
                nc.vector.tensor_tensor(var[:], e2[:], var[:], op=OP.subtract)
                nc.vector.tensor_scalar_add(var[:], var[:], EPS)
                sq = scp.tile([128, nch], f32, tag=f"sq{l}", name=f"sq{l}")
                nc.scalar.activation(sq[:], var[:], AF.Sqrt)
                y0 = scp.tile([128, nch], f32, tag=f"y0{l}", name=f"y0{l}")
                nc.vector.reciprocal(y0[:], sq[:])
                # one Newton step: y1 = y0 * (1.5 - 0.5 * var * y0^2)
                yy = scp.tile([128, nch], f32, tag=f"yy{l}", name=f"yy{l}")
                nc.vector.tensor_tensor(yy[:], y0[:], y0[:], op=OP.mult)
                nc.vector.tensor_tensor(yy[:], var[:], yy[:], op=OP.mult)
                nc.vector.tensor_scalar(yy[:], yy[:], -0.5, 1.5,
                                        op0=OP.mult, op1=OP.add)
                nc.vector.tensor_tensor(y0[:], y0[:], yy[:], op=OP.mult)
                nc.vector.tensor_tensor(sv[l][:, :nch], gp[:], y0[:], op=OP.mult)
                ms = scp.tile([128, nch], f32, tag=f"ms{l}", name=f"ms{l}")
                nc.vector.tensor_tensor(ms[:], mg[:], sv[l][:, :nch], op=OP.mult)
                nc.vector.tensor_tensor(tv[l][:, :nch], bp[:], ms[:], op=OP.subtract)
                if extra_bias is not None:
                    nc.vector.tensor_tensor(tv[l][:, :nch], tv[l][:, :nch],
                                            extra_bias[:], op=OP.add)

            def cast_hm(si, dst, src):
                # alternate engines so neither DVE nor ACT owns all casts
                if si % 2 == 0:
                    nc.vector.tensor_copy(dst, src)
                else:
                    nc.scalar.copy(dst, src)

            def mp_layer(st_slot0, gc, hm_list, nch_out, zsb, bias_pk, lsl):
                """c-outer message passing + bias + bn_stats for one supertile."""
                for c in range(nch_out):
                    for hi, (h0, hw, subs) in enumerate(_halves(gc)):
                        aggps = ps_s.tile([128, 512], f32, tag="small", name="aggps")
                        for (r0, rr, rp) in subs:
                            hm_sb = hm_list[[s[0] for s in _subtiles(gc)].index(r0)]
                            nc.tensor.matmul(aggps[:, r0 - h0:r0 - h0 + rr],
                                             lhsT=hm_sb[:rp, c * 128:(c + 1) * 128],
                                             rhs=abd_sb[:rp, :rr],
                                             start=True, stop=True)
                        nc.scalar.activation(zsb[c][:, h0:h0 + hw], aggps[:, :hw],
                                             AF.Identity, bias=bias_pk[:, c:c + 1])
                        slot = st_slot0 + hi
                        nc.vector.bn_stats(lsl[c][:, slot * 6:(slot + 1) * 6],
                                           zsb[c][:, h0:h0 + hw])

            # ================= Phase A: SE gate + layer 1 + xg spill ====
            with tc.tile_pool(name="wA", bufs=1) as wp, \
                 tc.tile_pool(name="actA", bufs=2) as ap_:
                w1_sb = []
                sew1_sb = []
                for k in range(4):
                    t = wp.tile([128, H1], f16, tag=f"w1_{k}", name=f"w1sb_{k}")
                    nc.sync.dma_start(t[:], w1_d.ap()[k * 128:(k + 1) * 128, :])
                    w1_sb.append(t)
                    t = wp.tile([128, SEH], f16, tag=f"sew1_{k}", name=f"sew1sb_{k}")
                    nc.sync.dma_start(t[:], sew1_d.ap()[k * 128:(k + 1) * 128, :])
                    sew1_sb.append(t)
                sew2_sb = wp.tile([128, CIN], f16, tag="sew2")
                nc.sync.dma_start(sew2_sb[:], sew2_d.ap()[:])

                st_slot = 0
                for sti, (g0, gc) in enumerate(sts):
                    rows = gc * N
                    c0 = g0 * N
                    x_sb = []
                    for k in range(4):
                        t = ap_.tile([128, STR], f16, tag=f"x_{k}", name=f"x_{k}")
                        nc.sync.dma_start(t[:, :rows],
                                          xT.ap()[k * 128:(k + 1) * 128, c0:c0 + rows])
                        x_sb.append(t)
                    # SE: node-sum (1/21 folded into sew1) -> 2 tiny mms
                    xm = []
                    for k in range(4):
                        t = ap_.tile([128, ST_G], f16, tag=f"xm_{k}", name=f"xm_{k}")
                        with nc.allow_low_precision(reason="fp16 rounding for SE matmul"):
                            nc.vector.tensor_reduce(
                                t[:, :gc],
                                x_sb[k][:, :rows].rearrange("p (g n) -> p g n", n=N),
                                axis=AX, op=OP.add)
                        xm.append(t)
                    y1ps = ps_s.tile([128, 512], f32, tag="small", name="y1ps")
                    for k in range(4):
                        nc.tensor.matmul(y1ps[:, :gc], lhsT=sew1_sb[k][:],
                                         rhs=xm[k][:, :gc],
                                         start=(k == 0), stop=(k == 3))
                    y1_sb = ap_.tile([128, ST_G], f16, tag="y1", name="y1")
                    nc.scalar.activation(y1_sb[:, :gc], y1ps[:, :gc], AF.Relu)
                    y_sb = []
                    for m in range(4):
                        y2ps = ps_s.tile([128, 512], f32, tag="small", name="y2ps")
                        nc.tensor.matmul(y2ps[:, :gc],
                                         lhsT=sew2_sb[:, m * 128:(m + 1) * 128],
                                         rhs=y1_sb[:, :gc], start=True, stop=True)
                        t = ap_.tile([128, ST_G], f16, tag=f"y_{m}", name=f"y_{m}")
                        nc.scalar.activation(t[:, :gc], y2ps[:, :gc], AF.Sigmoid)
                        y_sb.append(t)
                    # gate + xg spill
                    xg = []
                    for k in range(4):
                        t = ap_.tile([128, STR], f16, tag=f"xg_{k}", name=f"xg_{k}")
                        nc.vector.tensor_tensor(
                            t[:, :rows].rearrange("p (g n) -> p g n", n=N),
                            x_sb[k][:, :rows].rearrange("p (g n) -> p g n", n=N),
                            y_sb[k][:, :gc].broadcast_to([128, gc, N]),
                            op=OP.mult)
                        nc.sync.dma_start(xg_t[k * 128:(k + 1) * 128, c0:c0 + rows],
                                          t[:, :rows])
                        xg.append(t)
                    # layer-1 channel matmuls
                    hm_list = []
                    for si, (r0, rr, rp) in enumerate(_subtiles(gc)):
                        hmps = ps_hm.tile([128, H1], f32, tag="hm", name="hmps")
                        for k in range(4):
                            for n2 in range(2):
                                nc.tensor.matmul(
                                    hmps[:rp, n2 * 512:(n2 + 1) * 512],
                                    lhsT=xg[k][:, r0:r0 + rp],
                                    rhs=w1_sb[k][:, n2 * 512:(n2 + 1) * 512],
                                    start=(k == 0), stop=(k == 3))
                        hm_sb = ap_.tile([128, H1], f16, tag="hmsb", name="hm_sb",
                                         bufs=7)
                        cast_hm(si, hm_sb[:rp, :], hmps[:rp, :])
                        hm_list.append(hm_sb)
                    z1sb = [ap_.tile([128, STR], f16, tag=f"z1sb_{c}", name=f"z1sb_{c}")
                            for c in range(8)]
                    mp_layer(st_slot, gc, hm_list, 8, z1sb, pk["b1p"], bnsl[1])
                    for c in range(8):
                        nc.sync.dma_start(z1_t[c * 128:(c + 1) * 128, c0:c0 + rows],
                                          z1sb[c][:, :rows])
                    st_slot += len(_halves(gc))

                finalize_stats(1, 8, pk["bn1gp"], pk["bn1bp"], pk["rs1bp"])

            # ======== Phase B: junction 1 (+r1 mm from psum) + layer 2 ==
            with tc.tile_pool(name="wB", bufs=1) as wp, \
                 tc.tile_pool(name="actB", bufs=2) as ap_:
                w2_sb = []
                for k in range(8):
                    t = wp.tile([128, H2], f16, tag=f"w2_{k}", name=f"w2sb_{k}")
                    nc.sync.dma_start(t[:], w2_d.ap()[k * 128:(k + 1) * 128, :])
                    w2_sb.append(t)
                rs1_sb = []
                for k in range(4):
                    t = wp.tile([128, H1], f16, tag=f"rs1_{k}", name=f"rs1sb_{k}")
                    nc.sync.dma_start(t[:], rs1_d.ap()[k * 128:(k + 1) * 128, :])
                    rs1_sb.append(t)
                st_slot = 0
                for sti, (g0, gc) in enumerate(sts):
                    rows = gc * N
                    c0 = g0 * N
                    xgr = []
                    for k in range(4):
                        t = ap_.tile([128, STR], f16, tag=f"xgr_{k}", name=f"xgr_{k}")
                        nc.sync.dma_start(t[:, :rows],
                                          xg_t[k * 128:(k + 1) * 128, c0:c0 + rows])
                        xgr.append(t)
                    h1 = []
                    for c in range(8):
                        z1r = ap_.tile([128, STR], f16, tag=f"z1r_{c}", name=f"z1r_{c}")
                        nc.sync.dma_start(z1r[:, :rows],
                                          z1_t[c * 128:(c + 1) * 128, c0:c0 + rows])
                        ut = ap_.tile([128, STR], f16, tag=f"ut_{c}", name=f"ut_{c}")
                        hvs = _halves(gc)
                        r1ps_l = [ps_s.tile([128, 512], f32, tag="small",
                                            name=f"r1ps{hi}") for hi in range(len(hvs))]
                        for k in range(4):
                            for hi, (h0, hw, subs) in enumerate(hvs):
                                nc.tensor.matmul(r1ps_l[hi][:, :hw],
                                                 lhsT=rs1_sb[k][:, c * 128:(c + 1) * 128],
                                                 rhs=xgr[k][:, h0:h0 + hw],
                                                 start=(k == 0), stop=(k == 3))
                        for hi, (h0, hw, subs) in enumerate(hvs):
                            nc.vector.scalar_tensor_tensor(
                                ut[:, h0:h0 + hw], z1r[:, h0:h0 + hw],
                                sv[1][:, c:c + 1], r1ps_l[hi][:, :hw],
                                op0=OP.mult, op1=OP.add)
                        t = ap_.tile([128, STR], f16, tag=f"h1_{c}", name=f"h1_{c}")
                        nc.scalar.activation(t[:, :rows], ut[:, :rows], AF.Relu,
                                             bias=tv[1][:, c:c + 1])
                        nc.sync.dma_start(h1_t[c * 128:(c + 1) * 128, c0:c0 + rows],
                                          t[:, :rows])
                        h1.append(t)
                    hm_list = []
                    for si, (r0, rr, rp) in enumerate(_subtiles(gc)):
                        hmps = ps_hm.tile([128, H2], f32, tag="hm", name="hmps2")
                        for k in range(8):
                            for n2 in range(2):
                                nc.tensor.matmul(
                                    hmps[:rp, n2 * 512:(n2 + 1) * 512],
                                    lhsT=h1[k][:, r0:r0 + rp],
                                    rhs=w2_sb[k][:, n2 * 512:(n2 + 1) * 512],
                                    start=(k == 0), stop=(k == 7))
                        hm_sb = ap_.tile([128, H2], f16, tag="hmsb", name="hm_sb2",
                                         bufs=7)
                        cast_hm(si, hm_sb[:rp, :], hmps[:rp, :])
                        hm_list.append(hm_sb)
                    z2sb = [ap_.tile([128, STR], f16, tag=f"z2sb_{c}", name=f"z2sb_{c}")
                            for c in range(8)]
                    mp_layer(st_slot, gc, hm_list, 8, z2sb, pk["b2p"], bnsl[2])
                    for c in range(8):
                        nc.sync.dma_start(z2_t[c * 128:(c + 1) * 128, c0:c0 + rows],
                                          z2sb[c][:, :rows])
                    st_slot += len(_halves(gc))
                finalize_stats(2, 8, pk["bn2gp"], pk["bn2bp"])

            # ================= Phase C: junction 2 + layer 3 + pool =====
            with tc.tile_pool(name="wC", bufs=1) as wp, \
                 tc.tile_pool(name="actC", bufs=2) as ap_:
                w3_sb = []
                for k in range(8):
                    t = wp.tile([128, COUT], f16, tag=f"w3_{k}", name=f"w3sb_{k}")
                    nc.sync.dma_start(t[:], w3_d.ap()[k * 128:(k + 1) * 128, :])
                    w3_sb.append(t)
                st_slot = 0
                for sti, (g0, gc) in enumerate(sts):
                    rows = gc * N
                    c0 = g0 * N
                    h2 = []
                    for c in range(8):
                        z2r = ap_.tile([128, STR], f16, tag=f"z2r_{c}", name=f"z2r_{c}")
                        nc.sync.dma_start(z2r[:, :rows],
                                          z2_t[c * 128:(c + 1) * 128, c0:c0 + rows])
                        h1r = ap_.tile([128, STR], f16, tag=f"h1r_{c}", name=f"h1r_{c}")
                        nc.sync.dma_start(h1r[:, :rows],
                                          h1_t[c * 128:(c + 1) * 128, c0:c0 + rows])
                        bt = ap_.tile([128, STR], f16, tag=f"bt_{c}", name=f"bt_{c}")
                        nc.vector.scalar_tensor_tensor(
                            bt[:, :rows], z2r[:, :rows], sv[2][:, c:c + 1],
                            h1r[:, :rows], op0=OP.mult, op1=OP.add)
                        t = ap_.tile([128, STR], f16, tag=f"h2_{c}", name=f"h2_{c}")
                        nc.scalar.activation(t[:, :rows], bt[:, :rows], AF.Relu,
                                             bias=tv[2][:, c:c + 1])
                        h2.append(t)
                    hm_list = []
                    for si, (r0, rr, rp) in enumerate(_subtiles(gc)):
                        hmps = ps_hm.tile([128, H2], f32, tag="hm", name="hmps3")
                        for k in range(8):
                            nc.tensor.matmul(hmps[:rp, :COUT],
                                             lhsT=h2[k][:, r0:r0 + rp],
                                             rhs=w3_sb[k][:],
                                             start=(k == 0), stop=(k == 7))
                        hm_sb = ap_.tile([128, H2], f16, tag="hmsb", name="hm_sb3",
                                         bufs=7)
                        cast_hm(si, hm_sb[:rp, :COUT], hmps[:rp, :COUT])
                        hm_list.append(hm_sb)
                    z3sb = [ap_.tile([128, STR], f16, tag=f"z3sb_{c}", name=f"z3sb_{c}")
                            for c in range(4)]
                    mp_layer(st_slot, gc, hm_list, 4, z3sb, pk["b3p"], bnsl[3])
                    for c in range(4):
                        nc.vector.tensor_reduce(
                            pool_sb[c][:, g0:g0 + gc],
                            z3sb[c][:, :rows].rearrange("p (g n) -> p g n", n=N),
                            axis=AX, op=OP.add)
                    st_slot += len(_halves(gc))
                finalize_stats(3, 4, pk["bn3gp"], pk["bn3bp"])
                nc.vector.tensor_scalar_mul(s3p[:], sv[3][:], 1.0 / N)

            # ================= Phase D: pooled affine + FC ==============
            with tc.tile_pool(name="wD", bufs=1) as wp, \
                 tc.tile_pool(name="actD", bufs=2) as ap_:
                fcw_sb = []
                for k in range(4):
                    t = wp.tile([128, FCO], f16, tag=f"fcw_{k}", name=f"fcwsb_{k}")
                    nc.sync.dma_start(t[:], fcw_d.ap()[k * 128:(k + 1) * 128, :])
                    fcw_sb.append(t)
                pbn = []
                for c in range(4):
                    t = ap_.tile([128, gpc], f16, tag=f"pbn_{c}", name=f"pbn_{c}")
                    nc.vector.tensor_scalar(t[:], pool_sb[c][:],
                                            s3p[:, c:c + 1], tv[3][:, c:c + 1],
                                            op0=OP.mult, op1=OP.add)
                    pbn.append(t)
                for m in range(8):
                    fcps = ps_s.tile([128, 512], f32, tag="small", name="fcps")
                    for k in range(4):
                        nc.tensor.matmul(fcps[:, :gpc],
                                         lhsT=fcw_sb[k][:, m * 128:(m + 1) * 128],
                                         rhs=pbn[k][:],
                                         start=(k == 0), stop=(k == 3))
                    osb = ap_.tile([128, gpc], f32, tag="osb", name="osb")
                    nc.scalar.activation(osb[:], fcps[:, :gpc], AF.Identity,
                                         bias=pk["fcbp"][:, m:m + 1])
                    nc.sync.dma_start(out_d.ap()[m * 128:(m + 1) * 128, :], osb[:])

    _dedup_ldweights(nc, mybir)
    nc.compile()
    return nc


def host_prep(inputs, gpc=GPC, n_cores=NCORES):
    """Build per-core in_maps from the full problem inputs."""
    g = lambda k: np.asarray(inputs[k], np.float32)
    A = _host_adjacency(inputs["nodevec1"], inputs["nodevec2"])
    R = SUB_G * N
    abd = np.zeros((128, R), np.float32)
    for b in range(SUB_G):
        abd[b * N:(b + 1) * N, b * N:(b + 1) * N] = A.T   # rhs[j, i] = A[i, j]
    abd = abd.astype(np.float16)

    h = lambda k: np.asarray(inputs[k], np.float32).astype(np.float16)
    shared = {
        "w1": h("W1"), "rs1w": h("rs1_w"), "w2": h("W2"), "w3": h("W3"),
        "fcw": h("fc_w"),
        "sew1": (g("se_w1") / np.float32(N)).astype(np.float16),
        "sew2": h("se_w2"), "abd": abd,
        "b1p": _pack_ch(g("b1"), 8), "bn1gp": _pack_ch(g("bn1_g"), 8),
        "bn1bp": _pack_ch(g("bn1_b"), 8), "rs1bp": _pack_ch(g("rs1_b"), 8),
        "b2p": _pack_ch(g("b2"), 8), "bn2gp": _pack_ch(g("bn2_g"), 8),
        "bn2bp": _pack_ch(g("bn2_b"), 8),
        "b3p": _pack_ch(g("b3"), 4), "bn3gp": _pack_ch(g("bn3_g"), 4),
        "bn3bp": _pack_ch(g("bn3_b"), 4), "fcbp": _pack_ch(g("fc_b"), 8),
    }
    shared = {k: np.ascontiguousarray(v) for k, v in shared.items()}
    x = g("x")
    rows = gpc * N
    in_maps = []
    for i in range(n_cores):
        m = dict(shared)
        m["xT"] = np.ascontiguousarray(x[i * rows:(i + 1) * rows, :].T.astype(np.float16))
        in_maps.append(m)
    return in_maps


_cache = {}


def run(inputs, trace=False, trace_cores=None):
    from concourse.bass_utils import run_bass_kernel_spmd
    key = (GPC, NCORES)
    if key not in _cache:
        _cache[key] = build_nc(GPC, NCORES)
    nc = _cache[key]
    in_maps = host_prep(inputs, GPC, NCORES)
    res = run_bass_kernel_spmd(nc, in_maps, core_ids=list(range(NCORES)),
                               trace=trace, trace_cores=trace_cores)
    shards = [np.asarray(res.results[i]["out"]) for i in range(NCORES)]
    out = np.concatenate([s.T for s in shards], axis=0).astype(np.float32)
    return out, res


def kernel(**inputs) -> np.ndarray:
    out, _ = run(inputs, trace=False)
    return out
